# revision 1
# baseline (speedup 1.0000x reference)
"""Trainium2 Bass kernel for DUPN-style LSTM + windowed-softmax attention pooling.

Math (per batch element b):
  LSTM over T=128 steps (torch gate order), hidden H=512, input D=256.
  a[t] = sigmoid(x[t]·u1 + h[t]·u2), u1 = (v1@A1)^T, u2 = (v1@A2)^T  (folded)
  out[b,k,:] = softmax-pooled sum of h[t] over window t <= t_k, for 4 slots.

Sharding: data-parallel over batch, 32 per core x 8 cores, weights replicated.

Per-core device schedule:
  - xw = x@W_ih^T + bias precomputed in row-chunks of 128 rows (4 timesteps),
    fused into the loop as a prefetch, kept in an SBUF ring. Bias folded in
    via a K=1 ones-row matmul. Computed in two [128, 1024] halves to fit PSUM.
  - Per step: z [32, 2048] accumulated in a 4-bank PSUM tile: 4 identity
    matmuls inject xw rows (K=32), then 16 k-pass matmuls add h @ W_hh^T.
    All matmul outputs start at PSUM partition 0 (walrus emits col_grp=0xf
    only; non-zero dst partitions are unencodable).
  - Matmul operands are fp32r (fp32 rounded to 11 mantissa bits; full PE rate
    at N>=256). walrus requires producer dtype = fp32r, so matmul-feeding
    tiles are declared fp32r and written by converting copies.
  - Gates are free-dim slices of z (order i,f,o,g): sigmoid on [:, 0:1536],
    tanh on [:, 1536:2048]; c/h updates on DVE, everything at partition 0.
  - h transposed to hT via 4 PE transposes into hsT_store (fp32r), which is
    the next step's matmul stationary and the pooling source.
  - Post-loop: a = sigmoid(a1+a2), windowed softmax with host-built masks,
    pooling via per-b [4,T]@[T,H] matmuls.
"""
import sys

if "/opt/trn_rl_repo" not in sys.path:
    sys.path.insert(0, "/opt/trn_rl_repo")

import numpy as np
import concourse.bass as bass
import concourse.bacc as bacc
import concourse.tile as tile
from concourse import mybir
from concourse.bass_utils import run_bass_kernel_spmd
from contextlib import ExitStack

F32 = mybir.dt.float32
F32R = mybir.dt.float32r
AFT = mybir.ActivationFunctionType
ALU = mybir.AluOpType

T, BF, D, H, K, NC = 128, 256, 256, 512, 4, 8
BL = BF // NC          # 32 batch per core
G = 4 * H              # 2048
NEG_INF = -1e9

_cached = {}


def _build_program(t_steps=T):
    nc = bacc.Bacc()
    # ---- DRAM I/O (fp32r where feeding matmuls; same bytes as fp32) ----
    d_xT = nc.declare_dram_parameter("xT", [D, t_steps * BL], F32R, isOutput=False)
    d_wih = nc.declare_dram_parameter("wih", [D, G], F32R, isOutput=False)
    d_whh = nc.declare_dram_parameter("whh", [H, G], F32R, isOutput=False)
    d_biasrow = nc.declare_dram_parameter("biasrow", [1, G], F32R, isOutput=False)
    d_ones = nc.declare_dram_parameter("onesrow", [1, 128], F32R, isOutput=False)
    d_u1t = nc.declare_dram_parameter("u1t", [128, 2 * (D // 128)], F32R, isOutput=False)
    d_u2b = nc.declare_dram_parameter("u2b", [BL, H], F32, isOutput=False)
    d_i32s = nc.declare_dram_parameter("i32s", [128, 32], F32, isOutput=False)
    d_i128 = nc.declare_dram_parameter("i128", [128, 128], F32, isOutput=False)
    d_maskneg = nc.declare_dram_parameter("maskneg", [BL, K * t_steps], F32, isOutput=False)
    d_valid = nc.declare_dram_parameter("valid", [BL, K], F32, isOutput=False)
    d_out = nc.declare_dram_parameter("out", [BL * K, H], F32, isOutput=True)

    NRC = t_steps // 4     # row chunks of 128 rows (4 timesteps each)

    with tile.TileContext(nc) as tc, ExitStack() as ctx:
        nv, ns, nt, ng = nc.vector, nc.scalar, nc.tensor, nc.gpsimd

        consts = ctx.enter_context(tc.tile_pool(name="consts", bufs=1))
        big = ctx.enter_context(tc.tile_pool(name="big", bufs=1))

        # ---- load constants ----
        wih_sb = [consts.tile([128, G], F32R, tag=f"wih{i}", name=f"wih{i}")
                  for i in range(2)]
        for i in range(2):
            nc.sync.dma_start(wih_sb[i][:], d_wih[128 * i:128 * (i + 1), :])
        whh_sb = [consts.tile([128, G], F32R, tag=f"whh{i}", name=f"whh{i}")
                  for i in range(4)]
        for i in range(4):
            nc.sync.dma_start(whh_sb[i][:], d_whh[128 * i:128 * (i + 1), :])
        biasrow_sb = consts.tile([1, G], F32R, tag="biasrow")
        nc.sync.dma_start(biasrow_sb[:], d_biasrow[:])
        ones_sb = consts.tile([1, 128], F32R, tag="ones")
        nc.sync.dma_start(ones_sb[:], d_ones[:])
        u1t_sb = consts.tile([128, 4], F32R, tag="u1t")
        nc.sync.dma_start(u1t_sb[:], d_u1t[:])
        u2b_sb = consts.tile([BL, H], F32, tag="u2b")
        nc.sync.dma_start(u2b_sb[:], d_u2b[:])
        i32s_r = consts.tile([128, 32], F32R, tag="i32s_r")
        nc.sync.dma_start(i32s_r[:], d_i32s[:].bitcast(F32R))
        i32s_f = consts.tile([128, 32], F32, tag="i32s_f")
        nc.sync.dma_start(i32s_f[:], d_i32s[:])
        i128_r = consts.tile([128, 128], F32R, tag="i128_r")
        nc.sync.dma_start(i128_r[:], d_i128[:].bitcast(F32R))
        maskneg_sb = consts.tile([BL, K * t_steps], F32, tag="maskneg")
        nc.sync.dma_start(maskneg_sb[:], d_maskneg[:])
        valid_sb = consts.tile([BL, K], F32, tag="valid")
        nc.sync.dma_start(valid_sb[:], d_valid[:])

        # ---- persistent state ----
        hsT = big.tile([128, t_steps * 128], F32R, tag="hsT")      # [p, t*128+c*32+b]
        c_sb = big.tile([BL, H], F32, tag="c")
        a1ch = big.tile([128, NRC], F32, tag="a1ch")               # a1 by row-chunk
        a2_sb = big.tile([BL, t_steps], F32, tag="a2")

        # ---- loop pools ----
        loop_ctx = ExitStack()
        xt_pool = loop_ctx.enter_context(tc.tile_pool(name="xt", bufs=2))
        xw_pool = loop_ctx.enter_context(tc.tile_pool(name="xw", bufs=2))
        gate_pool = loop_ctx.enter_context(tc.tile_pool(name="gate", bufs=2))
        tmp_pool = loop_ctx.enter_context(tc.tile_pool(name="tmp", bufs=2))
        h_pool = loop_ctx.enter_context(tc.tile_pool(name="h", bufs=2))
        scr_pool = loop_ctx.enter_context(tc.tile_pool(name="scr", bufs=1))
        ps_xw = loop_ctx.enter_context(tc.tile_pool(name="ps_xw", bufs=1, space="PSUM"))
        ps_z = loop_ctx.enter_context(tc.tile_pool(name="ps_z", bufs=1, space="PSUM"))
        ps_hT = loop_ctx.enter_context(tc.tile_pool(name="ps_hT", bufs=1, space="PSUM"))
        ps_a1 = loop_ctx.enter_context(tc.tile_pool(name="ps_a1", bufs=1, space="PSUM"))

        def emit_xw_chunk(r):
            """xw rows 128r..128r+128 (timesteps 4r..4r+3) -> xw ring + a1 col r.

            Two [128, 1024] PSUM halves (2 banks each, bufs=2) to stay in
            budget: ps_xw 2x2 + ps_z 4 + ps_hT 1 + ps_a1 1 = 8 banks.
            """
            xtc = [xt_pool.tile([128, 128], F32R, tag=f"xtc{kd}", name=f"xtc{kd}_{r}")
                   for kd in range(2)]
            for kd in range(2):
                nc.sync.dma_start(xtc[kd][:],
                                  d_xT[128 * kd:128 * (kd + 1), 128 * r:128 * (r + 1)])
            xw = xw_pool.tile([128, G], F32R, tag="xw")
            for half in range(2):
                pxw = ps_xw.tile([128, 1024], F32, tag="pxw")
                for kd in range(2):
                    for n in range(2):
                        nn_ = 2 * half + n
                        nt.matmul(pxw[:, 512 * n:512 * (n + 1)], xtc[kd],
                                  wih_sb[kd][:, 512 * nn_:512 * (nn_ + 1)],
                                  start=(kd == 0), stop=False)
                for n in range(2):
                    nn_ = 2 * half + n
                    nt.matmul(pxw[:, 512 * n:512 * (n + 1)], ones_sb[:],
                              biasrow_sb[:, 512 * nn_:512 * (nn_ + 1)],
                              start=False, stop=True)
                if half == 0:
                    ns.copy(xw[:, 0:1024], pxw[:])
                else:
                    nv.tensor_copy(xw[:, 1024:2048], pxw[:])
            pa1 = ps_a1.tile([128, 2], F32)
            for kd in range(2):
                nt.matmul(pa1[:], xtc[kd], u1t_sb[:, 2 * kd:2 * kd + 2],
                          start=(kd == 0), stop=(kd == 1))
            ns.copy(a1ch[:, r:r + 1], pa1[:, 0:1])
            return xw

        xw_tiles = {0: emit_xw_chunk(0)}

        for t in range(t_steps):
            r, t4 = divmod(t, 4)
            xw = xw_tiles[r]
            pz = ps_z.tile([BL, G], F32, tag="pz")
            # n-chunk outer: chunk n finishes early so gates can start sooner
            for n in range(4):
                nt.matmul(pz[:, 512 * n:512 * (n + 1)],
                          i32s_r[32 * t4:32 * (t4 + 1), :],
                          xw[32 * t4:32 * (t4 + 1), 512 * n:512 * (n + 1)],
                          start=True, stop=(t == 0),
                          tile_position=(32 * t4, 0))
                if t > 0:
                    for k in range(4):
                        nt.matmul(
                            pz[:, 512 * n:512 * (n + 1)],
                            hsT[:, (t - 1) * 128 + 32 * k:(t - 1) * 128 + 32 * (k + 1)],
                            whh_sb[k][:, 512 * n:512 * (n + 1)],
                            start=False, stop=(k == 3))
            # gates: z cols [i(0:512) f(512:1024) o(1024:1536) g(1536:2048)]
            sg = gate_pool.tile([BL, 1536], F32, tag="sg")
            ns.activation(sg[:], pz[:, 0:1536], AFT.Sigmoid)
            gg = gate_pool.tile([BL, 512], F32, tag="gg")
            ns.activation(gg[:], pz[:, 1536:2048], AFT.Tanh)
            tig = tmp_pool.tile([BL, H], F32, tag="tig")
            nv.tensor_tensor(tig[:], sg[:, 0:512], gg[:], op=ALU.mult)
            if t == 0:
                nv.tensor_copy(c_sb[:], tig[:])
            else:
                tfc = tmp_pool.tile([BL, H], F32, tag="tfc")
                nv.tensor_tensor(tfc[:], sg[:, 512:1024], c_sb[:], op=ALU.mult)
                nv.tensor_tensor(c_sb[:], tfc[:], tig[:], op=ALU.add)
            tcs = tmp_pool.tile([BL, H], F32, tag="tcs")
            ns.activation(tcs[:], c_sb[:], AFT.Tanh)
            h_t = h_pool.tile([BL, H], F32, tag="h")
            nv.tensor_tensor(h_t[:], sg[:, 1024:1536], tcs[:], op=ALU.mult)
            # a2[t] = h . u2  (per-partition dot)
            scr = scr_pool.tile([BL, H], F32, tag="scr")
            nv.scalar_tensor_tensor(scr[:], h_t[:], 1.0, u2b_sb[:],
                                    op0=ALU.bypass, op1=ALU.mult,
                                    accum_out=a2_sb[:, t:t + 1])
            # transpose h -> hsT[:, t*128:(t+1)*128] (converts to fp32r)
            phT = ps_hT.tile([128, 128], F32, tag="phT")
            for c in range(4):
                nt.transpose(phT[:, 32 * c:32 * (c + 1)],
                             h_t[:, 128 * c:128 * (c + 1)], i32s_f[0:32, :])
            ns.copy(hsT[:, t * 128:(t + 1) * 128], phT[:])
            # prefetch next xw chunk (3 steps of slack before it's consumed)
            if t4 == 0 and r + 1 < NRC:
                xw_tiles[r + 1] = emit_xw_chunk(r + 1)
                xw_tiles.pop(r - 1, None)

        loop_ctx.close()

        # ---- post-loop: attention scores + softmax + pooling ----
        post = ctx.enter_context(tc.tile_pool(name="post", bufs=1))
        ps_t = ctx.enter_context(tc.tile_pool(name="ps_t", bufs=2, space="PSUM"))
        ps_pool = ctx.enter_context(tc.tile_pool(name="ps_pool", bufs=4, space="PSUM"))
        stg_pool = ctx.enter_context(tc.tile_pool(name="stg", bufs=4))
        hsb_pool = ctx.enter_context(tc.tile_pool(name="hsb", bufs=2))

        # a1 assembly: a1bp[b, 4r+c] = a1ch[32c+b, r]
        a1bp = post.tile([BL, t_steps], F32, tag="a1bp")
        for c in range(4):
            nv.tensor_copy(a1bp[:].rearrange("b (r c) -> b r c", c=4)[:, :, c],
                           a1ch[32 * c:32 * (c + 1), :])
        abp = post.tile([BL, t_steps], F32, tag="abp")
        nv.tensor_tensor(abp[:], a1bp[:], a2_sb[:], op=ALU.add)
        ns.activation(abp[:], abp[:], AFT.Sigmoid)

        # softmax per slot k -> wT [t, 4b+k] (fp32r for the pooling matmul)
        wT = post.tile([t_steps, K * BL], F32R, tag="wT")
        for k in range(K):
            sc = post.tile([BL, t_steps], F32, tag=f"sc{k}")
            nv.tensor_tensor(sc[:], abp[:],
                             maskneg_sb[:, t_steps * k:t_steps * (k + 1)], op=ALU.add)
            mneg = post.tile([BL, 1], F32, tag=f"mneg{k}")
            nv.tensor_reduce(mneg[:], sc[:], axis=mybir.AxisListType.X,
                             op=ALU.max, negate=True)
            ek = post.tile([BL, t_steps], F32, tag=f"ek{k}")
            sk = post.tile([BL, 1], F32, tag=f"sk{k}")
            ns.activation(ek[:], sc[:], AFT.Exp, bias=mneg[:], accum_out=sk[:])
            rk = post.tile([BL, 1], F32, tag=f"rk{k}")
            nv.reciprocal(rk[:], sk[:])
            wk = post.tile([BL, t_steps], F32, tag=f"wk{k}")
            nv.tensor_scalar(out=wk[:], in0=ek[:], scalar1=rk[:],
                             scalar2=valid_sb[:, k:k + 1], op0=ALU.mult, op1=ALU.mult)
            # transpose into wT columns k::4  (wT[t, 4b+k])
            pwT = ps_t.tile([128, 32], F32, tag="pwT")
            nt.transpose(pwT[0:t_steps, :], wk[:], i32s_f[0:32, :])
            nv.tensor_copy(wT[:].rearrange("t (b k) -> t b k", k=4)[:, :, k],
                           pwT[0:t_steps, :])

        # pooling: per b, rebuild hs_b [t, h] via 4 PE transposes, then [4,T]@[T,H]
        hsT_r = hsT[:].rearrange("p (t c b) -> p t c b", c=4, b=BL)
        for b in range(BL):
            hsb = hsb_pool.tile([t_steps, H], F32R, tag="hsb")
            for c in range(4):
                pt = ps_t.tile([128, 128], F32R, tag="pt")
                nt.transpose(pt[0:t_steps, :], hsT_r[:, :, c, b], i128_r[:])
                if c % 2 == 0:
                    ns.copy(hsb[:, 128 * c:128 * (c + 1)], pt[0:t_steps, :])
                else:
                    nv.tensor_copy(hsb[:, 128 * c:128 * (c + 1)], pt[0:t_steps, :])
            pp = ps_pool.tile([K, H], F32, tag="pp")
            nt.matmul(pp[:], wT[0:t_steps, 4 * b:4 * (b + 1)], hsb[:],
                      start=True, stop=True)
            so = stg_pool.tile([K, H], F32, tag="so")
            ns.copy(so[:], pp[:])
            nc.sync.dma_start(d_out[K * b:K * (b + 1), :], so[:])

    nc.compile()
    return nc


def _host_prep(x, W_ih, W_hh, b_ih, b_hh, A1, A2, v1, lengths, label_len):
    assert int(label_len) == K
    perm = np.concatenate([np.arange(0, 512), np.arange(512, 1024),
                           np.arange(1536, 2048), np.arange(1024, 1536)])
    wih = np.ascontiguousarray(W_ih[perm].T, dtype=np.float32)          # [256, 2048]
    whh = np.ascontiguousarray(W_hh[perm].T, dtype=np.float32)          # [512, 2048]
    biasrow = ((b_ih + b_hh)[perm]).astype(np.float32).reshape(1, G)
    u1 = (v1 @ A1)[0].astype(np.float32)                                # [256]
    u2 = (v1 @ A2)[0].astype(np.float32)                                # [512]
    u1t = np.zeros((128, 4), dtype=np.float32)                          # [128, 4]
    u1t[:, 0] = u1[0:128]
    u1t[:, 2] = u1[128:256]
    u2b = np.ascontiguousarray(np.broadcast_to(u2, (BL, H)))            # [32, 512]
    i32s = np.zeros((128, 32), dtype=np.float32)
    i32s[np.arange(128), np.arange(128) % 32] = 1.0
    i128 = np.eye(128, dtype=np.float32)

    shared = dict(wih=wih, whh=whh, biasrow=biasrow, u1t=u1t, u2b=u2b,
                  i32s=i32s, i128=i128, onesrow=np.ones((1, 128), dtype=np.float32))

    in_maps = []
    for cidx in range(NC):
        sl = slice(cidx * BL, (cidx + 1) * BL)
        xc = x[:, sl, :]                                                # [T, 32, D]
        xT = np.ascontiguousarray(xc.reshape(T * BL, D).T, dtype=np.float32)
        ln = lengths[sl].astype(np.int64)
        t_start = np.maximum(ln - K, 0)
        t_k = t_start[:, None] + np.arange(K)[None, :]                  # [32, 4]
        valid = (t_k <= (ln[:, None] - 1))                              # [32, 4]
        tt = np.arange(T)
        mask = (tt[None, None, :] <= t_k[:, :, None]) & valid[:, :, None]  # [b, k, t]
        maskneg = np.where(mask, 0.0, NEG_INF).astype(np.float32)
        maskneg = np.ascontiguousarray(maskneg.reshape(BL, K * T))      # k-major cols
        in_maps.append(dict(shared, xT=xT, maskneg=maskneg,
                            valid=valid.astype(np.float32)))
    return in_maps


def kernel(**inputs) -> np.ndarray:
    inputs = {k: np.asarray(v) if not np.isscalar(v) else v for k, v in inputs.items()}
    in_maps = _host_prep(**inputs)
    if "nc" not in _cached:
        _cached["nc"] = _build_program()
    nc = _cached["nc"]
    res = run_bass_kernel_spmd(nc, in_maps, core_ids=list(range(NC)))
    outs = []
    for cidx in range(NC):
        o = res.results[cidx]["out"]                                    # [128, 512]
        outs.append(o.reshape(BL, K, H))
    return np.concatenate(outs, axis=0).astype(np.float32)              # [256, 4, 512]



# revision 4
# speedup vs baseline: 1.2183x; 1.2183x over previous
"""Trainium2 Bass kernel for DUPN-style LSTM + windowed-softmax attention pooling.

Math (per batch element b):
  LSTM over T=128 steps (torch gate order), hidden H=512, input D=256.
  a[t] = sigmoid(x[t]·u1 + h[t]·u2), u1 = (v1@A1)^T, u2 = (v1@A2)^T  (folded)
  out[b,k,:] = softmax-pooled sum of h[t] over window t <= t_k, for 4 slots.

Sharding: data-parallel over batch, 32 per core x 8 cores, weights replicated.

Per-core schedule (v2 — pipelined tail, ~7.5us/step target):
  - z gate-column order is (g, i, f, o) so the c-update chain starts as soon
    as the first whh column chunks finish; per-gate [32,512] activations
    overlap the remaining whh matmuls.
  - h is kept ONLY in transposed form: sig(o) and c are PE-transposed
    separately into one PSUM tile, tanh runs on the 128-partition cT, and a
    DVE multiply writes hsT (fp32r) directly — no scalar-copy on the
    recurrence critical path.
  - xw = x@W_ih^T + bias is computed in per-step quarters ([128,512] PSUM,
    3 matmuls each) so the PE prefetch work is spread evenly; quarter
    copies alternate scalar/vector engines at queue tails.
  - a2[t] = h_t·u2 via 4 tiny stationary-reuse matmuls (N=2) during step
    t+1; a1 via u1 matmuls on the xw input chunks.
  - next-step xw injects (identity matmuls, start=True) are emitted between
    the whh block and the transposes to fill the PE stall while the tail
    chain runs, keeping the PE p-state warm.
  - PSUM: pz 4 banks + pxw 1 + psOC 1 + pa1 1 + pa2 1 = 8.
  - Post-loop: windowed softmax with host-built masks, pooling via per-b
    [4,T]@[T,H] matmuls on PE-transposed hs.
"""
import sys

if "/opt/trn_rl_repo" not in sys.path:
    sys.path.insert(0, "/opt/trn_rl_repo")

import numpy as np
import concourse.bass as bass
import concourse.bacc as bacc
import concourse.tile as tile
from concourse import mybir
from concourse.bass_utils import run_bass_kernel_spmd
from contextlib import ExitStack

F32 = mybir.dt.float32
F32R = mybir.dt.float32r
AFT = mybir.ActivationFunctionType
ALU = mybir.AluOpType

T, BF, D, H, K, NC = 128, 256, 256, 512, 4, 8
BL = BF // NC          # 32 batch per core
G = 4 * H              # 2048
NEG_INF = -1e9

_cached = {}


def _build_program(t_steps=T):
    nc = bacc.Bacc()
    # ---- DRAM I/O (fp32r where feeding matmuls; same bytes as fp32) ----
    d_xT = nc.declare_dram_parameter("xT", [D, t_steps * BL], F32R, isOutput=False)
    d_wih = nc.declare_dram_parameter("wih", [D, G], F32R, isOutput=False)
    d_whh = nc.declare_dram_parameter("whh", [H, G], F32R, isOutput=False)
    d_biasrow = nc.declare_dram_parameter("biasrow", [1, G], F32R, isOutput=False)
    d_ones = nc.declare_dram_parameter("onesrow", [1, 128], F32R, isOutput=False)
    d_u1t = nc.declare_dram_parameter("u1t", [128, 2 * (D // 128)], F32R, isOutput=False)
    d_u2t = nc.declare_dram_parameter("u2t", [128, 2 * (H // 128)], F32R, isOutput=False)
    d_i32s = nc.declare_dram_parameter("i32s", [128, 32], F32, isOutput=False)
    d_i128 = nc.declare_dram_parameter("i128", [128, 128], F32, isOutput=False)
    d_maskneg = nc.declare_dram_parameter("maskneg", [BL, K * t_steps], F32, isOutput=False)
    d_valid = nc.declare_dram_parameter("valid", [BL, K], F32, isOutput=False)
    d_out = nc.declare_dram_parameter("out", [BL * K, H], F32, isOutput=True)

    NRC = t_steps // 4     # row chunks of 128 rows (4 timesteps each)

    with tile.TileContext(nc) as tc, ExitStack() as ctx:
        nv, ns, nt = nc.vector, nc.scalar, nc.tensor

        consts = ctx.enter_context(tc.tile_pool(name="consts", bufs=1))
        big = ctx.enter_context(tc.tile_pool(name="big", bufs=1))

        # ---- load constants ----
        wih_sb = [consts.tile([128, G], F32R, tag=f"wih{i}", name=f"wih{i}")
                  for i in range(2)]
        for i in range(2):
            nc.sync.dma_start(wih_sb[i][:], d_wih[128 * i:128 * (i + 1), :])
        whh_sb = [consts.tile([128, G], F32R, tag=f"whh{i}", name=f"whh{i}")
                  for i in range(4)]
        for i in range(4):
            nc.sync.dma_start(whh_sb[i][:], d_whh[128 * i:128 * (i + 1), :])
        biasrow_sb = consts.tile([1, G], F32R, tag="biasrow")
        nc.sync.dma_start(biasrow_sb[:], d_biasrow[:])
        ones_sb = consts.tile([1, 128], F32R, tag="ones")
        nc.sync.dma_start(ones_sb[:], d_ones[:])
        u1t_sb = consts.tile([128, 4], F32R, tag="u1t")
        nc.sync.dma_start(u1t_sb[:], d_u1t[:])
        u2t_sb = consts.tile([128, 8], F32R, tag="u2t")
        nc.sync.dma_start(u2t_sb[:], d_u2t[:])
        i32s_r = consts.tile([128, 32], F32R, tag="i32s_r")
        nc.sync.dma_start(i32s_r[:], d_i32s[:].bitcast(F32R))
        i32s_f = consts.tile([128, 32], F32, tag="i32s_f")
        nc.sync.dma_start(i32s_f[:], d_i32s[:])
        i128_r = consts.tile([128, 128], F32R, tag="i128_r")
        nc.sync.dma_start(i128_r[:], d_i128[:].bitcast(F32R))
        maskneg_sb = consts.tile([BL, K * t_steps], F32, tag="maskneg")
        nc.sync.dma_start(maskneg_sb[:], d_maskneg[:])
        valid_sb = consts.tile([BL, K], F32, tag="valid")
        nc.sync.dma_start(valid_sb[:], d_valid[:])

        # ---- persistent state ----
        hsT = big.tile([128, t_steps * 128], F32R, tag="hsT")      # [p, t*128+c*32+b]
        c_sb = big.tile([BL, H], F32, tag="c")
        a1ch = big.tile([128, NRC], F32, tag="a1ch")               # a1 by row-chunk
        a2_sb = big.tile([BL, t_steps], F32, tag="a2")

        # ---- loop pools ----
        loop_ctx = ExitStack()
        xt_pool = loop_ctx.enter_context(tc.tile_pool(name="xt", bufs=2))
        xw_pool = loop_ctx.enter_context(tc.tile_pool(name="xw", bufs=2))
        gate_pool = loop_ctx.enter_context(tc.tile_pool(name="gate", bufs=2))
        tmp_pool = loop_ctx.enter_context(tc.tile_pool(name="tmp", bufs=2))
        tct_pool = loop_ctx.enter_context(tc.tile_pool(name="tct", bufs=2))
        ps_z = loop_ctx.enter_context(tc.tile_pool(name="ps_z", bufs=1, space="PSUM"))
        ps_xw = loop_ctx.enter_context(tc.tile_pool(name="ps_xw", bufs=1, space="PSUM"))
        ps_oc = loop_ctx.enter_context(tc.tile_pool(name="ps_oc", bufs=1, space="PSUM"))
        ps_a1 = loop_ctx.enter_context(tc.tile_pool(name="ps_a1", bufs=1, space="PSUM"))
        ps_a2 = loop_ctx.enter_context(tc.tile_pool(name="ps_a2", bufs=1, space="PSUM"))

        xw_tiles, xtc_tiles, pz_tiles = {}, {}, {}

        def emit_xw_quarter(rr, q):
            """PE matmuls for xw chunk rr, column quarter q -> pending PSUM.

            Returns (pxw, pa1): caller emits the PSUM->SBUF copies at its
            chosen queue positions. pa1 is non-None only at q==3.
            """
            if q == 0:
                xtc = [xt_pool.tile([128, 128], F32R, tag=f"xtc{kd}",
                                    name=f"xtc{kd}_{rr}") for kd in range(2)]
                for kd in range(2):
                    nc.sync.dma_start(
                        xtc[kd][:],
                        d_xT[128 * kd:128 * (kd + 1), 128 * rr:128 * (rr + 1)])
                xtc_tiles[rr] = xtc
                xw_tiles[rr] = xw_pool.tile([128, G], F32R, tag="xw",
                                            name=f"xw{rr}")
            xtc = xtc_tiles[rr]
            pxw = ps_xw.tile([128, 512], F32, tag="pxw")
            for kd in range(2):
                nt.matmul(pxw[:], xtc[kd],
                          wih_sb[kd][:, 512 * q:512 * (q + 1)],
                          start=(kd == 0), stop=False)
            nt.matmul(pxw[:], ones_sb[:], biasrow_sb[:, 512 * q:512 * (q + 1)],
                      start=False, stop=True)
            pa1 = None
            if q == 3:
                pa1 = ps_a1.tile([128, 2], F32, tag="pa1")
                for kd in range(2):
                    nt.matmul(pa1[:], xtc[kd], u1t_sb[:, 2 * kd:2 * kd + 2],
                              start=(kd == 0), stop=(kd == 1))
            return pxw, pa1

        def emit_injects(t):
            """Identity matmuls seeding pz[t] with xw rows (+bias)."""
            pz = ps_z.tile([BL, G], F32, tag="pz", name=f"pz{t}")
            pz_tiles[t] = pz
            rn, tn4 = divmod(t, 4)
            xwn = xw_tiles[rn]
            last = (t == 0)   # t=0 has no whh accumulation
            for n in range(4):
                nt.matmul(pz[:, 512 * n:512 * (n + 1)],
                          i32s_r[32 * tn4:32 * (tn4 + 1), :],
                          xwn[32 * tn4:32 * (tn4 + 1), 512 * n:512 * (n + 1)],
                          start=True, stop=last,
                          tile_position=(32 * tn4, 0))

        # ---- preamble: xw chunk 0 + injects for t=0 ----
        for q in range(4):
            pxw, pa1 = emit_xw_quarter(0, q)
            if q % 2 == 0:
                ns.copy(xw_tiles[0][:, 512 * q:512 * (q + 1)], pxw[:])
            else:
                nv.tensor_copy(xw_tiles[0][:, 512 * q:512 * (q + 1)], pxw[:])
            if pa1 is not None:
                ns.copy(a1ch[:, 0:1], pa1[:, 0:1])
        emit_injects(0)

        # gate column order: n0=g, n1=i, n2=f, n3=o (host perm matches)
        for t in range(t_steps):
            r, t4 = divmod(t, 4)
            pz = pz_tiles.pop(t)
            # --- PE: whh accumulation (n-outer so gate chunks finish early)
            if t > 0:
                for n in range(4):
                    for k in range(4):
                        nt.matmul(
                            pz[:, 512 * n:512 * (n + 1)],
                            hsT[:, (t - 1) * 128 + 32 * k:(t - 1) * 128 + 32 * (k + 1)],
                            whh_sb[k][:, 512 * n:512 * (n + 1)],
                            start=False, stop=(k == 3))
            # --- PE: a2 for t-1 (stationary = hsT chunks, tiny N)
            pa2 = None
            if t >= 1:
                pa2 = ps_a2.tile([BL, 2], F32, tag="pa2")
                for c4 in range(4):
                    nt.matmul(pa2[:],
                              hsT[:, (t - 1) * 128 + 32 * c4:(t - 1) * 128 + 32 * (c4 + 1)],
                              u2t_sb[:, 2 * c4:2 * c4 + 2],
                              start=(c4 == 0), stop=(c4 == 3))
            # --- PE: xw prefetch quarter for chunk r+1
            pxw_pend, pa1_pend = (None, None)
            if r + 1 < NRC:
                pxw_pend, pa1_pend = emit_xw_quarter(r + 1, t4)
            # --- ACT: per-gate activations (z cols [g | i | f | o])
            gg = gate_pool.tile([BL, 512], F32, tag="gg")
            ns.activation(gg[:], pz[:, 0:512], AFT.Tanh)
            si = gate_pool.tile([BL, 512], F32, tag="si")
            ns.activation(si[:], pz[:, 512:1024], AFT.Sigmoid)
            sf = gate_pool.tile([BL, 512], F32, tag="sf")
            if t > 0:
                ns.activation(sf[:], pz[:, 1024:1536], AFT.Sigmoid)
            so = gate_pool.tile([BL, 512], F32, tag="so")
            ns.activation(so[:], pz[:, 1536:2048], AFT.Sigmoid)
            # --- DVE: c update
            if t == 0:
                nv.tensor_tensor(c_sb[:], si[:], gg[:], op=ALU.mult)
            else:
                tig = tmp_pool.tile([BL, H], F32, tag="tig")
                nv.tensor_tensor(tig[:], si[:], gg[:], op=ALU.mult)
                tfc = tmp_pool.tile([BL, H], F32, tag="tfc")
                nv.tensor_tensor(tfc[:], sf[:], c_sb[:], op=ALU.mult)
                nv.tensor_tensor(c_sb[:], tfc[:], tig[:], op=ALU.add)
            # --- DVE: q3 xw copy early (inject n=3 of t+1 reads these cols,
            # so it must be EMITTED before emit_injects below)
            if pxw_pend is not None and t4 == 3:
                nv.tensor_copy(xw_tiles[r + 1][:, 1536:2048], pxw_pend[:])
                pxw_pend = None
            # --- PE: injects for t+1 (fill the tail stall, keep PE warm)
            if t + 1 < t_steps:
                emit_injects(t + 1)
            # --- PE: transposes of sig_o and c into one PSUM tile
            psOC = ps_oc.tile([128, 256], F32, tag="psOC")
            for c4 in range(4):
                nt.transpose(psOC[:, 32 * c4:32 * (c4 + 1)],
                             so[:, 128 * c4:128 * (c4 + 1)], i32s_f[0:32, :])
            for c4 in range(4):
                nt.transpose(psOC[:, 128 + 32 * c4:128 + 32 * (c4 + 1)],
                             c_sb[:, 128 * c4:128 * (c4 + 1)], i32s_f[0:32, :])
            # --- ACT: tanh on transposed c (128-partition, short free dim)
            tcT = tct_pool.tile([128, 128], F32, tag="tcT")
            ns.activation(tcT[:], psOC[:, 128:256], AFT.Tanh)
            # --- DVE: hsT = sig_oT * tanh_cT, split so whh k=0,1 start early
            for hh in range(2):
                nv.tensor_tensor(
                    hsT[:, t * 128 + 64 * hh:t * 128 + 64 * (hh + 1)],
                    psOC[:, 64 * hh:64 * (hh + 1)],
                    tcT[:, 64 * hh:64 * (hh + 1)], op=ALU.mult)
            # --- DVE: a2 copy-out
            if pa2 is not None:
                nv.tensor_copy(a2_sb[:, t - 1:t], pa2[:, 0:1])
            # --- queue-tail copies: xw quarter + a1
            if pxw_pend is not None:
                if t4 % 2 == 0:
                    ns.copy(xw_tiles[r + 1][:, 512 * t4:512 * (t4 + 1)],
                            pxw_pend[:])
                else:
                    nv.tensor_copy(xw_tiles[r + 1][:, 512 * t4:512 * (t4 + 1)],
                                   pxw_pend[:])
            if pa1_pend is not None:
                ns.copy(a1ch[:, r + 1:r + 2], pa1_pend[:, 0:1])
            xw_tiles.pop(r - 1, None)
            xtc_tiles.pop(r - 1, None)

        # --- a2 for the final step
        pa2 = ps_a2.tile([BL, 2], F32, tag="pa2")
        for c4 in range(4):
            nt.matmul(pa2[:],
                      hsT[:, (t_steps - 1) * 128 + 32 * c4:(t_steps - 1) * 128 + 32 * (c4 + 1)],
                      u2t_sb[:, 2 * c4:2 * c4 + 2],
                      start=(c4 == 0), stop=(c4 == 3))
        nv.tensor_copy(a2_sb[:, t_steps - 1:t_steps], pa2[:, 0:1])

        loop_ctx.close()

        # ---- post-loop: attention scores + softmax + pooling ----
        post = ctx.enter_context(tc.tile_pool(name="post", bufs=1))
        ps_t = ctx.enter_context(tc.tile_pool(name="ps_t", bufs=2, space="PSUM"))
        ps_pool = ctx.enter_context(tc.tile_pool(name="ps_pool", bufs=4, space="PSUM"))
        stg_pool = ctx.enter_context(tc.tile_pool(name="stg", bufs=4))
        hsb_pool = ctx.enter_context(tc.tile_pool(name="hsb", bufs=2))

        # a1 assembly: a1bp[b, 4r+c] = a1ch[32c+b, r]
        a1bp = post.tile([BL, t_steps], F32, tag="a1bp")
        for c in range(4):
            nv.tensor_copy(a1bp[:].rearrange("b (r c) -> b r c", c=4)[:, :, c],
                           a1ch[32 * c:32 * (c + 1), :])
        abp = post.tile([BL, t_steps], F32, tag="abp")
        nv.tensor_tensor(abp[:], a1bp[:], a2_sb[:], op=ALU.add)
        ns.activation(abp[:], abp[:], AFT.Sigmoid)

        # softmax per slot k -> wT [t, 4b+k] (fp32r for the pooling matmul)
        wT = post.tile([t_steps, K * BL], F32R, tag="wT")
        for k in range(K):
            sc = post.tile([BL, t_steps], F32, tag=f"sc{k}")
            nv.tensor_tensor(sc[:], abp[:],
                             maskneg_sb[:, t_steps * k:t_steps * (k + 1)], op=ALU.add)
            mneg = post.tile([BL, 1], F32, tag=f"mneg{k}")
            nv.tensor_reduce(mneg[:], sc[:], axis=mybir.AxisListType.X,
                             op=ALU.max, negate=True)
            ek = post.tile([BL, t_steps], F32, tag=f"ek{k}")
            sk = post.tile([BL, 1], F32, tag=f"sk{k}")
            ns.activation(ek[:], sc[:], AFT.Exp, bias=mneg[:], accum_out=sk[:])
            rk = post.tile([BL, 1], F32, tag=f"rk{k}")
            nv.reciprocal(rk[:], sk[:])
            wk = post.tile([BL, t_steps], F32, tag=f"wk{k}")
            nv.tensor_scalar(out=wk[:], in0=ek[:], scalar1=rk[:],
                             scalar2=valid_sb[:, k:k + 1], op0=ALU.mult, op1=ALU.mult)
            # transpose into wT columns k::4  (wT[t, 4b+k])
            pwT = ps_t.tile([128, 32], F32, tag="pwT")
            nt.transpose(pwT[0:t_steps, :], wk[:], i32s_f[0:32, :])
            nv.tensor_copy(wT[:].rearrange("t (b k) -> t b k", k=4)[:, :, k],
                           pwT[0:t_steps, :])

        # pooling: per b, rebuild hs_b [t, h] via 4 PE transposes, then [4,T]@[T,H]
        hsT_r = hsT[:].rearrange("p (t c b) -> p t c b", c=4, b=BL)
        for b in range(BL):
            hsb = hsb_pool.tile([t_steps, H], F32R, tag="hsb")
            for c in range(4):
                pt = ps_t.tile([128, 128], F32R, tag="pt")
                nt.transpose(pt[0:t_steps, :], hsT_r[:, :, c, b], i128_r[:])
                if c % 2 == 0:
                    ns.copy(hsb[:, 128 * c:128 * (c + 1)], pt[0:t_steps, :])
                else:
                    nv.tensor_copy(hsb[:, 128 * c:128 * (c + 1)], pt[0:t_steps, :])
            pp = ps_pool.tile([K, H], F32, tag="pp")
            nt.matmul(pp[:], wT[0:t_steps, 4 * b:4 * (b + 1)], hsb[:],
                      start=True, stop=True)
            so = stg_pool.tile([K, H], F32, tag="so")
            ns.copy(so[:], pp[:])
            nc.sync.dma_start(d_out[K * b:K * (b + 1), :], so[:])

    nc.compile()
    return nc


def _host_prep(x, W_ih, W_hh, b_ih, b_hh, A1, A2, v1, lengths, label_len):
    assert int(label_len) == K
    # torch gate rows (i,f,g,o) -> z column order (g,i,f,o)
    perm = np.concatenate([np.arange(1024, 1536), np.arange(0, 512),
                           np.arange(512, 1024), np.arange(1536, 2048)])
    wih = np.ascontiguousarray(W_ih[perm].T, dtype=np.float32)          # [256, 2048]
    whh = np.ascontiguousarray(W_hh[perm].T, dtype=np.float32)          # [512, 2048]
    biasrow = ((b_ih + b_hh)[perm]).astype(np.float32).reshape(1, G)
    u1 = (v1 @ A1)[0].astype(np.float32)                                # [256]
    u2 = (v1 @ A2)[0].astype(np.float32)                                # [512]
    u1t = np.zeros((128, 4), dtype=np.float32)                          # [128, 4]
    u1t[:, 0] = u1[0:128]
    u1t[:, 2] = u1[128:256]
    u2t = np.zeros((128, 8), dtype=np.float32)                          # [128, 8]
    for c in range(4):
        u2t[:, 2 * c] = u2[128 * c:128 * (c + 1)]
    i32s = np.zeros((128, 32), dtype=np.float32)
    i32s[np.arange(128), np.arange(128) % 32] = 1.0
    i128 = np.eye(128, dtype=np.float32)

    shared = dict(wih=wih, whh=whh, biasrow=biasrow, u1t=u1t, u2t=u2t,
                  i32s=i32s, i128=i128, onesrow=np.ones((1, 128), dtype=np.float32))

    in_maps = []
    for cidx in range(NC):
        sl = slice(cidx * BL, (cidx + 1) * BL)
        xc = x[:, sl, :]                                                # [T, 32, D]
        xT = np.ascontiguousarray(xc.reshape(T * BL, D).T, dtype=np.float32)
        ln = lengths[sl].astype(np.int64)
        t_start = np.maximum(ln - K, 0)
        t_k = t_start[:, None] + np.arange(K)[None, :]                  # [32, 4]
        valid = (t_k <= (ln[:, None] - 1))                              # [32, 4]
        tt = np.arange(T)
        mask = (tt[None, None, :] <= t_k[:, :, None]) & valid[:, :, None]  # [b, k, t]
        maskneg = np.where(mask, 0.0, NEG_INF).astype(np.float32)
        maskneg = np.ascontiguousarray(maskneg.reshape(BL, K * T))      # k-major cols
        in_maps.append(dict(shared, xT=xT, maskneg=maskneg,
                            valid=valid.astype(np.float32)))
    return in_maps


def kernel(**inputs) -> np.ndarray:
    inputs = {k: np.asarray(v) if not np.isscalar(v) else v for k, v in inputs.items()}
    in_maps = _host_prep(**inputs)
    if "nc" not in _cached:
        _cached["nc"] = _build_program()
    nc = _cached["nc"]
    res = run_bass_kernel_spmd(nc, in_maps, core_ids=list(range(NC)))
    outs = []
    for cidx in range(NC):
        o = res.results[cidx]["out"]                                    # [128, 512]
        outs.append(o.reshape(BL, K, H))
    return np.concatenate(outs, axis=0).astype(np.float32)              # [256, 4, 512]


# revision 7
# speedup vs baseline: 1.8315x; 1.5034x over previous
"""Trainium2 Bass kernel for DUPN-style LSTM + windowed-softmax attention pooling.

Math (per batch element b):
  LSTM over T=128 steps (torch gate order), hidden H=512, input D=256.
  a[t] = sigmoid(x[t]·u1 + h[t]·u2), u1 = (v1@A1)^T, u2 = (v1@A2)^T  (folded)
  out[b,k,:] = softmax-pooled sum of h[t] over window t <= t_k, for 4 slots.

Sharding: data-parallel over batch, 32 per core x 8 cores, weights replicated.

Per-core schedule (v2 — pipelined tail, ~7.5us/step target):
  - z gate-column order is (g, i, f, o) so the c-update chain starts as soon
    as the first whh column chunks finish; per-gate [32,512] activations
    overlap the remaining whh matmuls.
  - h is kept ONLY in transposed form: sig(o) and c are PE-transposed
    separately into one PSUM tile, tanh runs on the 128-partition cT, and a
    DVE multiply writes hsT (fp32r) directly — no scalar-copy on the
    recurrence critical path.
  - xw = x@W_ih^T + bias is computed in per-step quarters ([128,512] PSUM,
    3 matmuls each) so the PE prefetch work is spread evenly; quarter
    copies alternate scalar/vector engines at queue tails.
  - a2[t] = h_t·u2 via 4 tiny stationary-reuse matmuls (N=2) during step
    t+1; a1 via u1 matmuls on the xw input chunks.
  - next-step xw injects (identity matmuls, start=True) are emitted between
    the whh block and the transposes to fill the PE stall while the tail
    chain runs, keeping the PE p-state warm.
  - PSUM: pz 4 banks + pxw 1 + psOC 1 + pa1 1 + pa2 1 = 8.
  - Post-loop: windowed softmax with host-built masks, pooling via per-b
    [4,T]@[T,H] matmuls on PE-transposed hs.
"""
import sys

if "/opt/trn_rl_repo" not in sys.path:
    sys.path.insert(0, "/opt/trn_rl_repo")

import numpy as np
import concourse.bass as bass
import concourse.bacc as bacc
import concourse.tile as tile
from concourse import mybir
from concourse.bass_utils import run_bass_kernel_spmd
from contextlib import ExitStack

F32 = mybir.dt.float32
F32R = mybir.dt.float32r
AFT = mybir.ActivationFunctionType
ALU = mybir.AluOpType

T, BF, D, H, K, NC = 128, 256, 256, 512, 4, 8
BL = BF // NC          # 32 batch per core
G = 4 * H              # 2048
NEG_INF = -1e9

_cached = {}


def _build_program(t_steps=T):
    nc = bacc.Bacc()
    # ---- DRAM I/O (fp32r where feeding matmuls; same bytes as fp32) ----
    d_xT = nc.declare_dram_parameter("xT", [D, t_steps * BL], F32R, isOutput=False)
    d_wih = nc.declare_dram_parameter("wih", [D, G], F32R, isOutput=False)
    d_whh = nc.declare_dram_parameter("whh", [H, G], F32R, isOutput=False)
    d_biasrow = nc.declare_dram_parameter("biasrow", [1, G], F32R, isOutput=False)
    d_ones = nc.declare_dram_parameter("onesrow", [1, 128], F32R, isOutput=False)
    d_u1t = nc.declare_dram_parameter("u1t", [128, 2 * (D // 128)], F32R, isOutput=False)
    d_u2t = nc.declare_dram_parameter("u2t", [128, 2 * (H // 128)], F32R, isOutput=False)
    d_i32s = nc.declare_dram_parameter("i32s", [128, 32], F32, isOutput=False)
    d_i128 = nc.declare_dram_parameter("i128", [128, 128], F32, isOutput=False)
    d_maskneg = nc.declare_dram_parameter("maskneg", [BL, K * t_steps], F32, isOutput=False)
    d_valid = nc.declare_dram_parameter("valid", [BL, K], F32, isOutput=False)
    d_out = nc.declare_dram_parameter("out", [BL * K, H], F32, isOutput=True)

    NRC = t_steps // 4     # row chunks of 128 rows (4 timesteps each)

    with tile.TileContext(nc) as tc, ExitStack() as ctx:
        nv, ns, nt = nc.vector, nc.scalar, nc.tensor

        consts = ctx.enter_context(tc.tile_pool(name="consts", bufs=1))
        big = ctx.enter_context(tc.tile_pool(name="big", bufs=1))

        # ---- load constants ----
        wih_sb = [consts.tile([128, G], F32R, tag=f"wih{i}", name=f"wih{i}")
                  for i in range(2)]
        for i in range(2):
            nc.sync.dma_start(wih_sb[i][:], d_wih[128 * i:128 * (i + 1), :])
        whh_sb = [consts.tile([128, G], F32R, tag=f"whh{i}", name=f"whh{i}")
                  for i in range(4)]
        for i in range(4):
            nc.sync.dma_start(whh_sb[i][:], d_whh[128 * i:128 * (i + 1), :])
        biasrow_sb = consts.tile([1, G], F32R, tag="biasrow")
        nc.sync.dma_start(biasrow_sb[:], d_biasrow[:])
        ones_sb = consts.tile([1, 128], F32R, tag="ones")
        nc.sync.dma_start(ones_sb[:], d_ones[:])
        u1t_sb = consts.tile([128, 4], F32R, tag="u1t")
        nc.sync.dma_start(u1t_sb[:], d_u1t[:])
        u2t_sb = consts.tile([128, 8], F32R, tag="u2t")
        nc.sync.dma_start(u2t_sb[:], d_u2t[:])
        i32s_r = consts.tile([128, 32], F32R, tag="i32s_r")
        nc.sync.dma_start(i32s_r[:], d_i32s[:].bitcast(F32R))
        i32s_f = consts.tile([128, 32], F32, tag="i32s_f")
        nc.sync.dma_start(i32s_f[:], d_i32s[:])
        i128_r = consts.tile([128, 128], F32R, tag="i128_r")
        nc.sync.dma_start(i128_r[:], d_i128[:].bitcast(F32R))
        maskneg_sb = consts.tile([BL, K * t_steps], F32, tag="maskneg")
        nc.sync.dma_start(maskneg_sb[:], d_maskneg[:])
        valid_sb = consts.tile([BL, K], F32, tag="valid")
        nc.sync.dma_start(valid_sb[:], d_valid[:])

        # ---- persistent state ----
        hsT = big.tile([128, t_steps * 128], F32R, tag="hsT")      # [p, t*128+c*32+b]
        c_sb = big.tile([BL, H], F32, tag="c")
        a1ch = big.tile([128, NRC], F32, tag="a1ch")               # a1 by row-chunk
        a2_sb = big.tile([BL, t_steps], F32, tag="a2")

        # ---- loop pools ----
        loop_ctx = ExitStack()
        xt_pool = loop_ctx.enter_context(tc.tile_pool(name="xt", bufs=2))
        xw_pool = loop_ctx.enter_context(tc.tile_pool(name="xw", bufs=2))
        gate_pool = loop_ctx.enter_context(tc.tile_pool(name="gate", bufs=2))
        tmp_pool = loop_ctx.enter_context(tc.tile_pool(name="tmp", bufs=2))
        tct_pool = loop_ctx.enter_context(tc.tile_pool(name="tct", bufs=2))
        ps_z = loop_ctx.enter_context(tc.tile_pool(name="ps_z", bufs=1, space="PSUM"))
        ps_xw = loop_ctx.enter_context(tc.tile_pool(name="ps_xw", bufs=1, space="PSUM"))
        ps_oc = loop_ctx.enter_context(tc.tile_pool(name="ps_oc", bufs=1, space="PSUM"))
        ps_a1 = loop_ctx.enter_context(tc.tile_pool(name="ps_a1", bufs=1, space="PSUM"))
        ps_a2 = loop_ctx.enter_context(tc.tile_pool(name="ps_a2", bufs=1, space="PSUM"))

        xw_tiles, xtc_tiles, pz_tiles = {}, {}, {}

        def emit_xw_quarter(rr, q):
            """PE matmuls for xw chunk rr, column quarter q -> pending PSUM.

            Returns (pxw, pa1): caller emits the PSUM->SBUF copies at its
            chosen queue positions. pa1 is non-None only at q==3.
            """
            if q == 0:
                xtc = [xt_pool.tile([128, 128], F32R, tag=f"xtc{kd}",
                                    name=f"xtc{kd}_{rr}") for kd in range(2)]
                for kd in range(2):
                    nc.sync.dma_start(
                        xtc[kd][:],
                        d_xT[128 * kd:128 * (kd + 1), 128 * rr:128 * (rr + 1)])
                xtc_tiles[rr] = xtc
                xw_tiles[rr] = xw_pool.tile([128, G], F32R, tag="xw",
                                            name=f"xw{rr}")
            xtc = xtc_tiles[rr]
            pxw = ps_xw.tile([128, 512], F32, tag="pxw")
            for kd in range(2):
                nt.matmul(pxw[:], xtc[kd],
                          wih_sb[kd][:, 512 * q:512 * (q + 1)],
                          start=(kd == 0), stop=False)
            nt.matmul(pxw[:], ones_sb[:], biasrow_sb[:, 512 * q:512 * (q + 1)],
                      start=False, stop=True)
            pa1 = None
            if q == 3:
                pa1 = ps_a1.tile([128, 2], F32, tag="pa1")
                for kd in range(2):
                    nt.matmul(pa1[:], xtc[kd], u1t_sb[:, 2 * kd:2 * kd + 2],
                              start=(kd == 0), stop=(kd == 1))
            return pxw, pa1

        def emit_injects(t):
            """Identity matmuls seeding pz[t] with xw rows (+bias).

            One PSUM tile PER GATE so each bank's accumulation group closes
            independently — readers (per-gate activations) otherwise wait
            for the whole tile's group, serializing the tail after all whh.
            """
            pzs = [ps_z.tile([BL, 512], F32, tag=f"pz{n}", name=f"pz{n}_{t}")
                   for n in range(4)]
            pz_tiles[t] = pzs
            rn, tn4 = divmod(t, 4)
            xwn = xw_tiles[rn]
            last = (t == 0)   # t=0 has no whh accumulation
            for n in range(4):
                nt.matmul(pzs[n][:],
                          i32s_r[32 * tn4:32 * (tn4 + 1), :],
                          xwn[32 * tn4:32 * (tn4 + 1), 512 * n:512 * (n + 1)],
                          start=True, stop=last,
                          tile_position=(32 * tn4, 0))

        # ---- preamble: xw chunk 0 + injects for t=0 ----
        for q in range(4):
            pxw, pa1 = emit_xw_quarter(0, q)
            if q % 2 == 0:
                ns.copy(xw_tiles[0][:, 512 * q:512 * (q + 1)], pxw[:])
            else:
                nv.tensor_copy(xw_tiles[0][:, 512 * q:512 * (q + 1)], pxw[:])
            if pa1 is not None:
                ns.copy(a1ch[:, 0:1], pa1[:, 0:1])
        emit_injects(0)

        # gate column order: n0=g, n1=i, n2=f, n3=o (host perm matches)
        for t in range(t_steps):
            r, t4 = divmod(t, 4)
            pzs = pz_tiles.pop(t)
            # --- PE: whh accumulation (n-outer so gate chunks finish early)
            if t > 0:
                for n in range(4):
                    for k in range(4):
                        nt.matmul(
                            pzs[n][:],
                            hsT[:, (t - 1) * 128 + 32 * k:(t - 1) * 128 + 32 * (k + 1)],
                            whh_sb[k][:, 512 * n:512 * (n + 1)],
                            start=False, stop=(k == 3))
            # --- PE: a2 for t-1 (stationary = hsT chunks, tiny N)
            pa2 = None
            if t >= 1:
                pa2 = ps_a2.tile([BL, 2], F32, tag="pa2")
                for c4 in range(4):
                    nt.matmul(pa2[:],
                              hsT[:, (t - 1) * 128 + 32 * c4:(t - 1) * 128 + 32 * (c4 + 1)],
                              u2t_sb[:, 2 * c4:2 * c4 + 2],
                              start=(c4 == 0), stop=(c4 == 3))
            # --- PE: xw prefetch quarter for chunk r+1
            pxw_pend, pa1_pend = (None, None)
            if r + 1 < NRC:
                pxw_pend, pa1_pend = emit_xw_quarter(r + 1, t4)
            # --- ACT: per-gate activations (z cols [g | i | f | o])
            gg = gate_pool.tile([BL, 512], F32, tag="gg")
            ns.activation(gg[:], pzs[0][:], AFT.Tanh)
            si = gate_pool.tile([BL, 512], F32, tag="si")
            ns.activation(si[:], pzs[1][:], AFT.Sigmoid)
            sf = gate_pool.tile([BL, 512], F32, tag="sf")
            ns.activation(sf[:], pzs[2][:], AFT.Sigmoid)
            so = gate_pool.tile([BL, 512], F32, tag="so")
            ns.activation(so[:], pzs[3][:], AFT.Sigmoid)
            # --- DVE: c update
            if t == 0:
                nv.tensor_tensor(c_sb[:], si[:], gg[:], op=ALU.mult)
            else:
                tig = tmp_pool.tile([BL, H], F32, tag="tig")
                nv.tensor_tensor(tig[:], si[:], gg[:], op=ALU.mult)
                tfc = tmp_pool.tile([BL, H], F32, tag="tfc")
                nv.tensor_tensor(tfc[:], sf[:], c_sb[:], op=ALU.mult)
                nv.tensor_tensor(c_sb[:], tfc[:], tig[:], op=ALU.add)
            # --- DVE: q3 xw copy early (inject n=3 of t+1 reads these cols,
            # so it must be EMITTED before emit_injects below)
            if pxw_pend is not None and t4 == 3:
                nv.tensor_copy(xw_tiles[r + 1][:, 1536:2048], pxw_pend[:])
                pxw_pend = None
            # --- PE: injects for t+1 (fill the tail stall, keep PE warm)
            if t + 1 < t_steps:
                emit_injects(t + 1)
            # --- PE: transposes of sig_o and c into one PSUM tile
            psOC = ps_oc.tile([128, 256], F32, tag="psOC")
            for c4 in range(4):
                nt.transpose(psOC[:, 32 * c4:32 * (c4 + 1)],
                             so[:, 128 * c4:128 * (c4 + 1)], i32s_f[0:32, :])
            for c4 in range(4):
                nt.transpose(psOC[:, 128 + 32 * c4:128 + 32 * (c4 + 1)],
                             c_sb[:, 128 * c4:128 * (c4 + 1)], i32s_f[0:32, :])
            # --- ACT: tanh on transposed c (128-partition, short free dim)
            tcT = tct_pool.tile([128, 128], F32, tag="tcT")
            ns.activation(tcT[:], psOC[:, 128:256], AFT.Tanh)
            # --- DVE: hsT = sig_oT * tanh_cT, split so whh k=0,1 start early
            for hh in range(2):
                nv.tensor_tensor(
                    hsT[:, t * 128 + 64 * hh:t * 128 + 64 * (hh + 1)],
                    psOC[:, 64 * hh:64 * (hh + 1)],
                    tcT[:, 64 * hh:64 * (hh + 1)], op=ALU.mult)
            # --- DVE: a2 copy-out
            if pa2 is not None:
                nv.tensor_copy(a2_sb[:, t - 1:t], pa2[:, 0:1])
            # --- queue-tail copies: xw quarter + a1
            if pxw_pend is not None:
                if t4 % 2 == 0:
                    ns.copy(xw_tiles[r + 1][:, 512 * t4:512 * (t4 + 1)],
                            pxw_pend[:])
                else:
                    nv.tensor_copy(xw_tiles[r + 1][:, 512 * t4:512 * (t4 + 1)],
                                   pxw_pend[:])
            if pa1_pend is not None:
                ns.copy(a1ch[:, r + 1:r + 2], pa1_pend[:, 0:1])
            xw_tiles.pop(r - 1, None)
            xtc_tiles.pop(r - 1, None)

        # --- a2 for the final step
        pa2 = ps_a2.tile([BL, 2], F32, tag="pa2")
        for c4 in range(4):
            nt.matmul(pa2[:],
                      hsT[:, (t_steps - 1) * 128 + 32 * c4:(t_steps - 1) * 128 + 32 * (c4 + 1)],
                      u2t_sb[:, 2 * c4:2 * c4 + 2],
                      start=(c4 == 0), stop=(c4 == 3))
        nv.tensor_copy(a2_sb[:, t_steps - 1:t_steps], pa2[:, 0:1])

        loop_ctx.close()

        # ---- post-loop: attention scores + softmax + pooling ----
        post = ctx.enter_context(tc.tile_pool(name="post", bufs=1))
        ps_t = ctx.enter_context(tc.tile_pool(name="ps_t", bufs=2, space="PSUM"))
        ps_pool = ctx.enter_context(tc.tile_pool(name="ps_pool", bufs=4, space="PSUM"))
        stg_pool = ctx.enter_context(tc.tile_pool(name="stg", bufs=4))
        hsb_pool = ctx.enter_context(tc.tile_pool(name="hsb", bufs=2))

        # a1 assembly: a1bp[b, 4r+c] = a1ch[32c+b, r]
        a1bp = post.tile([BL, t_steps], F32, tag="a1bp")
        for c in range(4):
            nv.tensor_copy(a1bp[:].rearrange("b (r c) -> b r c", c=4)[:, :, c],
                           a1ch[32 * c:32 * (c + 1), :])
        abp = post.tile([BL, t_steps], F32, tag="abp")
        nv.tensor_tensor(abp[:], a1bp[:], a2_sb[:], op=ALU.add)
        ns.activation(abp[:], abp[:], AFT.Sigmoid)

        # softmax per slot k -> wT [t, 4b+k] (fp32r for the pooling matmul)
        wT = post.tile([t_steps, K * BL], F32R, tag="wT")
        for k in range(K):
            sc = post.tile([BL, t_steps], F32, tag=f"sc{k}")
            nv.tensor_tensor(sc[:], abp[:],
                             maskneg_sb[:, t_steps * k:t_steps * (k + 1)], op=ALU.add)
            mneg = post.tile([BL, 1], F32, tag=f"mneg{k}")
            nv.tensor_reduce(mneg[:], sc[:], axis=mybir.AxisListType.X,
                             op=ALU.max, negate=True)
            ek = post.tile([BL, t_steps], F32, tag=f"ek{k}")
            sk = post.tile([BL, 1], F32, tag=f"sk{k}")
            ns.activation(ek[:], sc[:], AFT.Exp, bias=mneg[:], accum_out=sk[:])
            rk = post.tile([BL, 1], F32, tag=f"rk{k}")
            nv.reciprocal(rk[:], sk[:])
            wk = post.tile([BL, t_steps], F32, tag=f"wk{k}")
            nv.tensor_scalar(out=wk[:], in0=ek[:], scalar1=rk[:],
                             scalar2=valid_sb[:, k:k + 1], op0=ALU.mult, op1=ALU.mult)
            # transpose into wT columns k::4  (wT[t, 4b+k])
            pwT = ps_t.tile([128, 32], F32, tag="pwT")
            nt.transpose(pwT[0:t_steps, :], wk[:], i32s_f[0:32, :])
            nv.tensor_copy(wT[:].rearrange("t (b k) -> t b k", k=4)[:, :, k],
                           pwT[0:t_steps, :])

        # pooling: per b, rebuild hs_b [t, h] via 4 PE transposes, then [4,T]@[T,H]
        hsT_r = hsT[:].rearrange("p (t c b) -> p t c b", c=4, b=BL)
        for b in range(BL):
            hsb = hsb_pool.tile([t_steps, H], F32R, tag="hsb")
            for c in range(4):
                pt = ps_t.tile([128, 128], F32R, tag="pt")
                nt.transpose(pt[0:t_steps, :], hsT_r[:, :, c, b], i128_r[:])
                if c % 2 == 0:
                    ns.copy(hsb[:, 128 * c:128 * (c + 1)], pt[0:t_steps, :])
                else:
                    nv.tensor_copy(hsb[:, 128 * c:128 * (c + 1)], pt[0:t_steps, :])
            pp = ps_pool.tile([K, H], F32, tag="pp")
            nt.matmul(pp[:], wT[0:t_steps, 4 * b:4 * (b + 1)], hsb[:],
                      start=True, stop=True)
            so = stg_pool.tile([K, H], F32, tag="so")
            ns.copy(so[:], pp[:])
            nc.sync.dma_start(d_out[K * b:K * (b + 1), :], so[:])

    nc.compile()
    return nc


def _host_prep(x, W_ih, W_hh, b_ih, b_hh, A1, A2, v1, lengths, label_len):
    assert int(label_len) == K
    # torch gate rows (i,f,g,o) -> z column order (g,i,f,o)
    perm = np.concatenate([np.arange(1024, 1536), np.arange(0, 512),
                           np.arange(512, 1024), np.arange(1536, 2048)])
    wih = np.ascontiguousarray(W_ih[perm].T, dtype=np.float32)          # [256, 2048]
    whh = np.ascontiguousarray(W_hh[perm].T, dtype=np.float32)          # [512, 2048]
    biasrow = ((b_ih + b_hh)[perm]).astype(np.float32).reshape(1, G)
    u1 = (v1 @ A1)[0].astype(np.float32)                                # [256]
    u2 = (v1 @ A2)[0].astype(np.float32)                                # [512]
    u1t = np.zeros((128, 4), dtype=np.float32)                          # [128, 4]
    u1t[:, 0] = u1[0:128]
    u1t[:, 2] = u1[128:256]
    u2t = np.zeros((128, 8), dtype=np.float32)                          # [128, 8]
    for c in range(4):
        u2t[:, 2 * c] = u2[128 * c:128 * (c + 1)]
    i32s = np.zeros((128, 32), dtype=np.float32)
    i32s[np.arange(128), np.arange(128) % 32] = 1.0
    i128 = np.eye(128, dtype=np.float32)

    shared = dict(wih=wih, whh=whh, biasrow=biasrow, u1t=u1t, u2t=u2t,
                  i32s=i32s, i128=i128, onesrow=np.ones((1, 128), dtype=np.float32))

    in_maps = []
    for cidx in range(NC):
        sl = slice(cidx * BL, (cidx + 1) * BL)
        xc = x[:, sl, :]                                                # [T, 32, D]
        xT = np.ascontiguousarray(xc.reshape(T * BL, D).T, dtype=np.float32)
        ln = lengths[sl].astype(np.int64)
        t_start = np.maximum(ln - K, 0)
        t_k = t_start[:, None] + np.arange(K)[None, :]                  # [32, 4]
        valid = (t_k <= (ln[:, None] - 1))                              # [32, 4]
        tt = np.arange(T)
        mask = (tt[None, None, :] <= t_k[:, :, None]) & valid[:, :, None]  # [b, k, t]
        maskneg = np.where(mask, 0.0, NEG_INF).astype(np.float32)
        maskneg = np.ascontiguousarray(maskneg.reshape(BL, K * T))      # k-major cols
        in_maps.append(dict(shared, xT=xT, maskneg=maskneg,
                            valid=valid.astype(np.float32)))
    return in_maps


def kernel(**inputs) -> np.ndarray:
    inputs = {k: np.asarray(v) if not np.isscalar(v) else v for k, v in inputs.items()}
    in_maps = _host_prep(**inputs)
    if "nc" not in _cached:
        _cached["nc"] = _build_program()
    nc = _cached["nc"]
    res = run_bass_kernel_spmd(nc, in_maps, core_ids=list(range(NC)))
    outs = []
    for cidx in range(NC):
        o = res.results[cidx]["out"]                                    # [128, 512]
        outs.append(o.reshape(BL, K, H))
    return np.concatenate(outs, axis=0).astype(np.float32)              # [256, 4, 512]


# revision 16
# speedup vs baseline: 1.9907x; 1.0869x over previous
"""Trainium2 Bass kernel for DUPN-style LSTM + windowed-softmax attention pooling.

Math (per batch element b):
  LSTM over T=128 steps (torch gate order), hidden H=512, input D=256.
  a[t] = sigmoid(x[t]·u1 + h[t]·u2), u1 = (v1@A1)^T, u2 = (v1@A2)^T  (folded)
  out[b,k,:] = softmax-pooled sum of h[t] over window t <= t_k, for 4 slots.

Sharding: data-parallel over batch, 32 per core x 8 cores, weights replicated.

Per-core schedule (v2 — pipelined tail, ~7.5us/step target):
  - z gate-column order is (g, i, f, o) so the c-update chain starts as soon
    as the first whh column chunks finish; per-gate [32,512] activations
    overlap the remaining whh matmuls.
  - h is kept ONLY in transposed form: sig(o) and c are PE-transposed
    separately into one PSUM tile, tanh runs on the 128-partition cT, and a
    DVE multiply writes hsT (fp32r) directly — no scalar-copy on the
    recurrence critical path.
  - xw = x@W_ih^T + bias is computed in per-step quarters ([128,512] PSUM,
    3 matmuls each) so the PE prefetch work is spread evenly; quarter
    copies alternate scalar/vector engines at queue tails.
  - a2[t] = h_t·u2 via 4 tiny stationary-reuse matmuls (N=2) during step
    t+1; a1 via u1 matmuls on the xw input chunks.
  - next-step xw injects (identity matmuls, start=True) are emitted between
    the whh block and the transposes to fill the PE stall while the tail
    chain runs, keeping the PE p-state warm.
  - PSUM: pz 4 banks + pxw 1 + psOC 1 + pa1 1 + pa2 1 = 8.
  - Post-loop: windowed softmax with host-built masks, pooling via per-b
    [4,T]@[T,H] matmuls on PE-transposed hs.
"""
import sys

if "/opt/trn_rl_repo" not in sys.path:
    sys.path.insert(0, "/opt/trn_rl_repo")

import numpy as np
import ml_dtypes
import concourse.bass as bass
import concourse.bacc as bacc
import concourse.tile as tile
from concourse import mybir
from concourse.bass_utils import run_bass_kernel_spmd
from contextlib import ExitStack

F32 = mybir.dt.float32
F32R = mybir.dt.float32r
F8E4 = mybir.dt.float8e4
U8 = mybir.dt.uint8
AFT = mybir.ActivationFunctionType
ALU = mybir.AluOpType
DROW = mybir.MatmulPerfMode.DoubleRow

T, BF, D, H, K, NC = 128, 256, 256, 512, 4, 8
BL = BF // NC          # 32 batch per core
G = 4 * H              # 2048
NEG_INF = -1e9

_cached = {}


def _build_program(t_steps=T):
    nc = bacc.Bacc()
    # ---- DRAM I/O (fp32r where feeding matmuls; same bytes as fp32) ----
    d_xT = nc.declare_dram_parameter("xT", [D, t_steps * BL], F32R, isOutput=False)
    d_wih = nc.declare_dram_parameter("wih", [D, G], F32R, isOutput=False)
    # W_hh^T in fp8e4, DoubleRow pair layout [p, pair, ksub, n] flattened
    d_whh8 = nc.declare_dram_parameter("whh8", [128, 2 * 2 * G], U8, isOutput=False)
    d_biasrow = nc.declare_dram_parameter("biasrow", [1, G], F32R, isOutput=False)
    d_ones = nc.declare_dram_parameter("onesrow", [1, 128], F32R, isOutput=False)
    d_u1t = nc.declare_dram_parameter("u1t", [128, 2 * (D // 128)], F32R, isOutput=False)
    d_u2t = nc.declare_dram_parameter("u2t", [128, 2 * (H // 128)], F32R, isOutput=False)
    d_i32s = nc.declare_dram_parameter("i32s", [128, 32], F32, isOutput=False)
    d_i128 = nc.declare_dram_parameter("i128", [128, 128], F32, isOutput=False)
    d_maskneg = nc.declare_dram_parameter("maskneg", [BL, K * t_steps], F32, isOutput=False)
    d_valid = nc.declare_dram_parameter("valid", [BL, K], F32, isOutput=False)
    d_out = nc.declare_dram_parameter("out", [BL * K, H], F32, isOutput=True)

    NRC = t_steps // 4     # row chunks of 128 rows (4 timesteps each)

    with tile.TileContext(nc) as tc, ExitStack() as ctx:
        nv, ns, nt = nc.vector, nc.scalar, nc.tensor

        consts = ctx.enter_context(tc.tile_pool(name="consts", bufs=1))
        big = ctx.enter_context(tc.tile_pool(name="big", bufs=1))

        # ---- load constants ----
        wih_sb = [consts.tile([128, G], F32R, tag=f"wih{i}", name=f"wih{i}")
                  for i in range(2)]
        for i in range(2):
            nc.sync.dma_start(wih_sb[i][:], d_wih[128 * i:128 * (i + 1), :])
        whh8_sb = consts.tile([128, 2 * 2 * G], F8E4, tag="whh8")
        nc.sync.dma_start(whh8_sb[:], d_whh8[:].bitcast(F8E4))
        whh8_r = whh8_sb[:].rearrange("p (pr ks n) -> p pr ks n", pr=2, ks=2)
        biasrow_sb = consts.tile([1, G], F32R, tag="biasrow")
        nc.sync.dma_start(biasrow_sb[:], d_biasrow[:])
        ones_sb = consts.tile([1, 128], F32R, tag="ones")
        nc.sync.dma_start(ones_sb[:], d_ones[:])
        u1t_sb = consts.tile([128, 4], F32R, tag="u1t")
        nc.sync.dma_start(u1t_sb[:], d_u1t[:])
        u2t_sb = consts.tile([128, 8], F32R, tag="u2t")
        nc.sync.dma_start(u2t_sb[:], d_u2t[:])
        i32s_r = consts.tile([128, 32], F32R, tag="i32s_r")
        nc.sync.dma_start(i32s_r[:], d_i32s[:].bitcast(F32R))
        i32s_f = consts.tile([128, 32], F32, tag="i32s_f")
        nc.sync.dma_start(i32s_f[:], d_i32s[:])
        i128_r = consts.tile([128, 128], F32R, tag="i128_r")
        nc.sync.dma_start(i128_r[:], d_i128[:].bitcast(F32R))
        maskneg_sb = consts.tile([BL, K * t_steps], F32, tag="maskneg")
        nc.sync.dma_start(maskneg_sb[:], d_maskneg[:])
        valid_sb = consts.tile([BL, K], F32, tag="valid")
        nc.sync.dma_start(valid_sb[:], d_valid[:])

        # ---- persistent state ----
        hsT = big.tile([128, t_steps * 128], F32R, tag="hsT")      # [p, t*128+c*32+b]
        c_sb = big.tile([BL, H], F32, tag="c")
        a1ch = big.tile([128, NRC], F32, tag="a1ch")               # a1 by row-chunk
        a2_sb = big.tile([BL, t_steps], F32, tag="a2")

        # ---- loop pools ----
        loop_ctx = ExitStack()
        xt_pool = loop_ctx.enter_context(tc.tile_pool(name="xt", bufs=2))
        xw_pool = loop_ctx.enter_context(tc.tile_pool(name="xw", bufs=2))
        gate_pool = loop_ctx.enter_context(tc.tile_pool(name="gate", bufs=2))
        tmp_pool = loop_ctx.enter_context(tc.tile_pool(name="tmp", bufs=2))
        tct_pool = loop_ctx.enter_context(tc.tile_pool(name="tct", bufs=2))
        h8_pool = loop_ctx.enter_context(tc.tile_pool(name="h8", bufs=2))
        ps_z = loop_ctx.enter_context(tc.tile_pool(name="ps_z", bufs=1, space="PSUM"))
        ps_xw = loop_ctx.enter_context(tc.tile_pool(name="ps_xw", bufs=1, space="PSUM"))
        ps_oc = loop_ctx.enter_context(tc.tile_pool(name="ps_oc", bufs=1, space="PSUM"))
        ps_a1 = loop_ctx.enter_context(tc.tile_pool(name="ps_a1", bufs=1, space="PSUM"))
        ps_a2 = loop_ctx.enter_context(tc.tile_pool(name="ps_a2", bufs=1, space="PSUM"))

        xw_tiles, xtc_tiles, pz_tiles = {}, {}, {}

        def emit_xw_quarter(rr, q):
            """PE matmuls for xw chunk rr, column quarter q -> pending PSUM.

            Returns (pxw, pa1): caller emits the PSUM->SBUF copies at its
            chosen queue positions. pa1 is non-None only at q==3.
            """
            if q == 0:
                xtc = [xt_pool.tile([128, 128], F32R, tag=f"xtc{kd}",
                                    name=f"xtc{kd}_{rr}") for kd in range(2)]
                for kd in range(2):
                    nc.sync.dma_start(
                        xtc[kd][:],
                        d_xT[128 * kd:128 * (kd + 1), 128 * rr:128 * (rr + 1)])
                xtc_tiles[rr] = xtc
                xw_tiles[rr] = xw_pool.tile([128, G], F32R, tag="xw",
                                            name=f"xw{rr}")
            xtc = xtc_tiles[rr]
            pxw = ps_xw.tile([128, 512], F32, tag="pxw")
            for kd in range(2):
                nt.matmul(pxw[:], xtc[kd],
                          wih_sb[kd][:, 512 * q:512 * (q + 1)],
                          start=(kd == 0), stop=False)
            nt.matmul(pxw[:], ones_sb[:], biasrow_sb[:, 512 * q:512 * (q + 1)],
                      start=False, stop=True)
            pa1 = None
            if q == 3:
                pa1 = ps_a1.tile([128, 2], F32, tag="pa1")
                for kd in range(2):
                    nt.matmul(pa1[:], xtc[kd], u1t_sb[:, 2 * kd:2 * kd + 2],
                              start=(kd == 0), stop=(kd == 1))
            return pxw, pa1

        def emit_injects(t):
            """Identity matmuls seeding pz[t] with xw rows (+bias).

            One PSUM tile PER GATE so each bank's accumulation group closes
            independently — readers (per-gate activations) otherwise wait
            for the whole tile's group, serializing the tail after all whh.
            """
            pzs = [ps_z.tile([BL, 512], F32, tag=f"pz{n}", name=f"pz{n}_{t}")
                   for n in range(4)]
            pz_tiles[t] = pzs
            rn, tn4 = divmod(t, 4)
            xwn = xw_tiles[rn]
            last = (t == 0)   # t=0 has no whh accumulation
            for n in range(4):
                nt.matmul(pzs[n][:],
                          i32s_r[32 * tn4:32 * (tn4 + 1), :],
                          xwn[32 * tn4:32 * (tn4 + 1), 512 * n:512 * (n + 1)],
                          start=True, stop=last,
                          tile_position=(32 * tn4, 0))

        # ---- preamble: xw chunk 0 + injects for t=0 ----
        for q in range(4):
            pxw, pa1 = emit_xw_quarter(0, q)
            if q % 2 == 0:
                ns.copy(xw_tiles[0][:, 512 * q:512 * (q + 1)], pxw[:])
            else:
                nv.tensor_copy(xw_tiles[0][:, 512 * q:512 * (q + 1)], pxw[:])
            if pa1 is not None:
                ns.copy(a1ch[:, 0:1], pa1[:, 0:1])
        emit_injects(0)

        # gate column order: n0=g, n1=i, n2=f, n3=o (host perm matches)
        for t in range(t_steps):
            r, t4 = divmod(t, 4)
            pzs = pz_tiles.pop(t)
            # --- PE: whh accumulation, fp8 DoubleRow (K=256 per pass),
            # n-outer so gate chunks finish early
            if t > 0:
                h8p = h8_prev[:].rearrange("p (pr ks b) -> p pr ks b",
                                           pr=2, ks=2)
                for n in range(4):
                    for pr in range(2):
                        nt.matmul(
                            pzs[n][:], h8p[:, pr],
                            whh8_r[:, pr, :, 512 * n:512 * (n + 1)],
                            start=False, stop=(pr == 1), perf_mode=DROW)
            # --- PE: a2 for t-1 (stationary = hsT chunks, tiny N)
            pa2 = None
            if t >= 1:
                pa2 = ps_a2.tile([BL, 2], F32, tag="pa2")
                for c4 in range(4):
                    nt.matmul(pa2[:],
                              hsT[:, (t - 1) * 128 + 32 * c4:(t - 1) * 128 + 32 * (c4 + 1)],
                              u2t_sb[:, 2 * c4:2 * c4 + 2],
                              start=(c4 == 0), stop=(c4 == 3))
            # --- PE: xw prefetch quarter for chunk r+1
            pxw_pend, pa1_pend = (None, None)
            if r + 1 < NRC:
                pxw_pend, pa1_pend = emit_xw_quarter(r + 1, t4)
            # --- ACT: per-gate activations (z cols [g | i | f | o])
            gg = gate_pool.tile([BL, 512], F32, tag="gg")
            ns.activation(gg[:], pzs[0][:], AFT.Tanh)
            si = gate_pool.tile([BL, 512], F32, tag="si")
            ns.activation(si[:], pzs[1][:], AFT.Sigmoid)
            sf = gate_pool.tile([BL, 512], F32, tag="sf")
            ns.activation(sf[:], pzs[2][:], AFT.Sigmoid)
            so = gate_pool.tile([BL, 512], F32, tag="so")
            ns.activation(so[:], pzs[3][:], AFT.Sigmoid)
            # --- DVE: c update
            if t == 0:
                nv.tensor_tensor(c_sb[:], si[:], gg[:], op=ALU.mult)
            else:
                tig = tmp_pool.tile([BL, H], F32, tag="tig")
                nv.tensor_tensor(tig[:], si[:], gg[:], op=ALU.mult)
                tfc = tmp_pool.tile([BL, H], F32, tag="tfc")
                nv.tensor_tensor(tfc[:], sf[:], c_sb[:], op=ALU.mult)
                nv.tensor_tensor(c_sb[:], tfc[:], tig[:], op=ALU.add)
            # --- DVE: q3 xw copy early (inject n=3 of t+1 reads these cols,
            # so it must be EMITTED before emit_injects below)
            if pxw_pend is not None and t4 == 3:
                nv.tensor_copy(xw_tiles[r + 1][:, 1536:2048], pxw_pend[:])
                pxw_pend = None
            # --- PE: transposes of sig_o and c into one PSUM tile
            psOC = ps_oc.tile([128, 256], F32, tag="psOC")
            for c4 in range(4):
                nt.transpose(psOC[:, 32 * c4:32 * (c4 + 1)],
                             so[:, 128 * c4:128 * (c4 + 1)], i32s_f[0:32, :])
            for c4 in range(4):
                nt.transpose(psOC[:, 128 + 32 * c4:128 + 32 * (c4 + 1)],
                             c_sb[:, 128 * c4:128 * (c4 + 1)], i32s_f[0:32, :])
            # --- PE: injects for t+1 (fill the tail stall, keep PE warm)
            if t + 1 < t_steps:
                emit_injects(t + 1)
            # --- ACT: tanh on transposed c (128-partition, short free dim)
            tcT = tct_pool.tile([128, 128], F32, tag="tcT")
            ns.activation(tcT[:], psOC[:, 128:256], AFT.Tanh)
            # --- DVE: h8 = fp8(sig_oT * tanh_cT) halves feed next whh first,
            # then the fp32r hsT copy for pooling/a2 (off the critical path)
            h8 = h8_pool.tile([128, 128], F8E4, tag="h8")
            for hh in range(2):
                nv.tensor_tensor(
                    h8[:, 64 * hh:64 * (hh + 1)],
                    psOC[:, 64 * hh:64 * (hh + 1)],
                    tcT[:, 64 * hh:64 * (hh + 1)], op=ALU.mult)
            h8_prev = h8
            nv.tensor_tensor(hsT[:, t * 128:(t + 1) * 128],
                             psOC[:, 0:128], tcT[:], op=ALU.mult)
            # --- DVE: a2 copy-out
            if pa2 is not None:
                nv.tensor_copy(a2_sb[:, t - 1:t], pa2[:, 0:1])
            # --- queue-tail copies: xw quarter + a1
            if pxw_pend is not None:
                if t4 % 2 == 0:
                    ns.copy(xw_tiles[r + 1][:, 512 * t4:512 * (t4 + 1)],
                            pxw_pend[:])
                else:
                    nv.tensor_copy(xw_tiles[r + 1][:, 512 * t4:512 * (t4 + 1)],
                                   pxw_pend[:])
            if pa1_pend is not None:
                ns.copy(a1ch[:, r + 1:r + 2], pa1_pend[:, 0:1])
            xw_tiles.pop(r - 1, None)
            xtc_tiles.pop(r - 1, None)

        # --- a2 for the final step
        pa2 = ps_a2.tile([BL, 2], F32, tag="pa2")
        for c4 in range(4):
            nt.matmul(pa2[:],
                      hsT[:, (t_steps - 1) * 128 + 32 * c4:(t_steps - 1) * 128 + 32 * (c4 + 1)],
                      u2t_sb[:, 2 * c4:2 * c4 + 2],
                      start=(c4 == 0), stop=(c4 == 3))
        nv.tensor_copy(a2_sb[:, t_steps - 1:t_steps], pa2[:, 0:1])

        loop_ctx.close()

        # ---- post-loop: attention scores + softmax + pooling ----
        post = ctx.enter_context(tc.tile_pool(name="post", bufs=1))
        ps_t = ctx.enter_context(tc.tile_pool(name="ps_t", bufs=2, space="PSUM"))
        ps_pool = ctx.enter_context(tc.tile_pool(name="ps_pool", bufs=4, space="PSUM"))
        stg_pool = ctx.enter_context(tc.tile_pool(name="stg", bufs=4))
        hsb_pool = ctx.enter_context(tc.tile_pool(name="hsb", bufs=2))

        # a1 assembly: a1bp[b, 4r+c] = a1ch[32c+b, r]
        a1bp = post.tile([BL, t_steps], F32, tag="a1bp")
        for c in range(4):
            nv.tensor_copy(a1bp[:].rearrange("b (r c) -> b r c", c=4)[:, :, c],
                           a1ch[32 * c:32 * (c + 1), :])
        abp = post.tile([BL, t_steps], F32, tag="abp")
        nv.tensor_tensor(abp[:], a1bp[:], a2_sb[:], op=ALU.add)
        ns.activation(abp[:], abp[:], AFT.Sigmoid)

        # softmax per slot k -> wT [t, 4b+k] (fp32r for the pooling matmul)
        wT = post.tile([t_steps, K * BL], F32R, tag="wT")
        for k in range(K):
            sc = post.tile([BL, t_steps], F32, tag=f"sc{k}")
            nv.tensor_tensor(sc[:], abp[:],
                             maskneg_sb[:, t_steps * k:t_steps * (k + 1)], op=ALU.add)
            mneg = post.tile([BL, 1], F32, tag=f"mneg{k}")
            nv.tensor_reduce(mneg[:], sc[:], axis=mybir.AxisListType.X,
                             op=ALU.max, negate=True)
            ek = post.tile([BL, t_steps], F32, tag=f"ek{k}")
            sk = post.tile([BL, 1], F32, tag=f"sk{k}")
            ns.activation(ek[:], sc[:], AFT.Exp, bias=mneg[:], accum_out=sk[:])
            rk = post.tile([BL, 1], F32, tag=f"rk{k}")
            nv.reciprocal(rk[:], sk[:])
            wk = post.tile([BL, t_steps], F32, tag=f"wk{k}")
            nv.tensor_scalar(out=wk[:], in0=ek[:], scalar1=rk[:],
                             scalar2=valid_sb[:, k:k + 1], op0=ALU.mult, op1=ALU.mult)
            # transpose into wT columns k::4  (wT[t, 4b+k])
            pwT = ps_t.tile([128, 32], F32, tag="pwT")
            nt.transpose(pwT[0:t_steps, :], wk[:], i32s_f[0:32, :])
            nv.tensor_copy(wT[:].rearrange("t (b k) -> t b k", k=4)[:, :, k],
                           pwT[0:t_steps, :])

        # pooling: per b, rebuild hs_b [t, h] via 4 PE transposes, then [4,T]@[T,H]
        hsT_r = hsT[:].rearrange("p (t c b) -> p t c b", c=4, b=BL)
        for b in range(BL):
            hsb = hsb_pool.tile([t_steps, H], F32R, tag="hsb")
            for c in range(4):
                pt = ps_t.tile([128, 128], F32R, tag="pt")
                nt.transpose(pt[0:t_steps, :], hsT_r[:, :, c, b], i128_r[:])
                if c % 2 == 0:
                    ns.copy(hsb[:, 128 * c:128 * (c + 1)], pt[0:t_steps, :])
                else:
                    nv.tensor_copy(hsb[:, 128 * c:128 * (c + 1)], pt[0:t_steps, :])
            pp = ps_pool.tile([K, H], F32, tag="pp")
            nt.matmul(pp[:], wT[0:t_steps, 4 * b:4 * (b + 1)], hsb[:],
                      start=True, stop=True)
            so = stg_pool.tile([K, H], F32, tag="so")
            ns.copy(so[:], pp[:])
            nc.sync.dma_start(d_out[K * b:K * (b + 1), :], so[:])

    nc.compile()
    return nc


def _host_prep(x, W_ih, W_hh, b_ih, b_hh, A1, A2, v1, lengths, label_len):
    assert int(label_len) == K
    # torch gate rows (i,f,g,o) -> z column order (g,i,f,o)
    perm = np.concatenate([np.arange(1024, 1536), np.arange(0, 512),
                           np.arange(512, 1024), np.arange(1536, 2048)])
    wih = np.ascontiguousarray(W_ih[perm].T, dtype=np.float32)          # [256, 2048]
    whhT = np.asarray(W_hh[perm].T, dtype=np.float32)                   # [512, 2048]
    # fp8 DoubleRow pair layout: whh8[p, pr, ks, n] = WhhT[256*pr+128*ks+p, n]
    whh8 = whhT.reshape(2, 2, 128, G).transpose(2, 0, 1, 3).reshape(128, 4 * G)
    whh8 = np.ascontiguousarray(whh8).astype(ml_dtypes.float8_e4m3).view(np.uint8)
    biasrow = ((b_ih + b_hh)[perm]).astype(np.float32).reshape(1, G)
    u1 = (v1 @ A1)[0].astype(np.float32)                                # [256]
    u2 = (v1 @ A2)[0].astype(np.float32)                                # [512]
    u1t = np.zeros((128, 4), dtype=np.float32)                          # [128, 4]
    u1t[:, 0] = u1[0:128]
    u1t[:, 2] = u1[128:256]
    u2t = np.zeros((128, 8), dtype=np.float32)                          # [128, 8]
    for c in range(4):
        u2t[:, 2 * c] = u2[128 * c:128 * (c + 1)]
    i32s = np.zeros((128, 32), dtype=np.float32)
    i32s[np.arange(128), np.arange(128) % 32] = 1.0
    i128 = np.eye(128, dtype=np.float32)

    shared = dict(wih=wih, whh8=whh8, biasrow=biasrow, u1t=u1t, u2t=u2t,
                  i32s=i32s, i128=i128, onesrow=np.ones((1, 128), dtype=np.float32))

    in_maps = []
    for cidx in range(NC):
        sl = slice(cidx * BL, (cidx + 1) * BL)
        xc = x[:, sl, :]                                                # [T, 32, D]
        xT = np.ascontiguousarray(xc.reshape(T * BL, D).T, dtype=np.float32)
        ln = lengths[sl].astype(np.int64)
        t_start = np.maximum(ln - K, 0)
        t_k = t_start[:, None] + np.arange(K)[None, :]                  # [32, 4]
        valid = (t_k <= (ln[:, None] - 1))                              # [32, 4]
        tt = np.arange(T)
        mask = (tt[None, None, :] <= t_k[:, :, None]) & valid[:, :, None]  # [b, k, t]
        maskneg = np.where(mask, 0.0, NEG_INF).astype(np.float32)
        maskneg = np.ascontiguousarray(maskneg.reshape(BL, K * T))      # k-major cols
        in_maps.append(dict(shared, xT=xT, maskneg=maskneg,
                            valid=valid.astype(np.float32)))
    return in_maps


def kernel(**inputs) -> np.ndarray:
    inputs = {k: np.asarray(v) if not np.isscalar(v) else v for k, v in inputs.items()}
    in_maps = _host_prep(**inputs)
    if "nc" not in _cached:
        _cached["nc"] = _build_program()
    nc = _cached["nc"]
    res = run_bass_kernel_spmd(nc, in_maps, core_ids=list(range(NC)))
    outs = []
    for cidx in range(NC):
        o = res.results[cidx]["out"]                                    # [128, 512]
        outs.append(o.reshape(BL, K, H))
    return np.concatenate(outs, axis=0).astype(np.float32)              # [256, 4, 512]


# revision 22
# speedup vs baseline: 2.0074x; 1.0084x over previous
"""Trainium2 Bass kernel for DUPN-style LSTM + windowed-softmax attention pooling.

Math (per batch element b):
  LSTM over T=128 steps (torch gate order), hidden H=512, input D=256.
  a[t] = sigmoid(x[t]·u1 + h[t]·u2), u1 = (v1@A1)^T, u2 = (v1@A2)^T  (folded)
  out[b,k,:] = softmax-pooled sum of h[t] over window t <= t_k, for 4 slots.

Sharding: data-parallel over batch, 32 per core x 8 cores, weights replicated.

Per-core schedule (v2 — pipelined tail, ~7.5us/step target):
  - z gate-column order is (g, i, f, o) so the c-update chain starts as soon
    as the first whh column chunks finish; per-gate [32,512] activations
    overlap the remaining whh matmuls.
  - h is kept ONLY in transposed form: sig(o) and c are PE-transposed
    separately into one PSUM tile, tanh runs on the 128-partition cT, and a
    DVE multiply writes hsT (fp32r) directly — no scalar-copy on the
    recurrence critical path.
  - xw = x@W_ih^T + bias is computed in per-step quarters ([128,512] PSUM,
    3 matmuls each) so the PE prefetch work is spread evenly; quarter
    copies alternate scalar/vector engines at queue tails.
  - a2[t] = h_t·u2 via 4 tiny stationary-reuse matmuls (N=2) during step
    t+1; a1 via u1 matmuls on the xw input chunks.
  - next-step xw injects (identity matmuls, start=True) are emitted between
    the whh block and the transposes to fill the PE stall while the tail
    chain runs, keeping the PE p-state warm.
  - PSUM: pz 4 banks + pxw 1 + psOC 1 + pa1 1 + pa2 1 = 8.
  - Post-loop: windowed softmax with host-built masks, pooling via per-b
    [4,T]@[T,H] matmuls on PE-transposed hs.
"""
import sys

if "/opt/trn_rl_repo" not in sys.path:
    sys.path.insert(0, "/opt/trn_rl_repo")

import numpy as np
import ml_dtypes
import concourse.bass as bass
import concourse.bacc as bacc
import concourse.tile as tile
from concourse import mybir
from concourse.bass_utils import run_bass_kernel_spmd
from contextlib import ExitStack

F32 = mybir.dt.float32
F32R = mybir.dt.float32r
F8E4 = mybir.dt.float8e4
U8 = mybir.dt.uint8
AFT = mybir.ActivationFunctionType
ALU = mybir.AluOpType
DROW = mybir.MatmulPerfMode.DoubleRow

T, BF, D, H, K, NC = 128, 256, 256, 512, 4, 8
BL = BF // NC          # 32 batch per core
G = 4 * H              # 2048
NEG_INF = -1e9

_cached = {}


def _build_program(t_steps=T):
    nc = bacc.Bacc()
    # ---- DRAM I/O (fp32r where feeding matmuls; same bytes as fp32) ----
    d_xT = nc.declare_dram_parameter("xT", [D, t_steps * BL], F32R, isOutput=False)
    d_wih = nc.declare_dram_parameter("wih", [D, G], F32R, isOutput=False)
    # W_hh^T in fp8e4, DoubleRow pair layout [p, pair, ksub, n] flattened
    d_whh8 = nc.declare_dram_parameter("whh8", [128, 2 * 2 * G], U8, isOutput=False)
    d_biasrow = nc.declare_dram_parameter("biasrow", [1, G], F32R, isOutput=False)
    d_ones = nc.declare_dram_parameter("onesrow", [1, 128], F32R, isOutput=False)
    d_u1t = nc.declare_dram_parameter("u1t", [128, 2 * (D // 128)], F32R, isOutput=False)
    d_u2t = nc.declare_dram_parameter("u2t", [128, 2 * (H // 128)], F32R, isOutput=False)
    d_i32s = nc.declare_dram_parameter("i32s", [128, 32], F32, isOutput=False)
    d_i128 = nc.declare_dram_parameter("i128", [128, 128], F32, isOutput=False)
    d_maskneg = nc.declare_dram_parameter("maskneg", [BL, K * t_steps], F32, isOutput=False)
    d_valid = nc.declare_dram_parameter("valid", [BL, K], F32, isOutput=False)
    d_out = nc.declare_dram_parameter("out", [BL * K, H], F32, isOutput=True)

    NRC = t_steps // 4     # row chunks of 128 rows (4 timesteps each)

    with tile.TileContext(nc) as tc, ExitStack() as ctx:
        nv, ns, nt = nc.vector, nc.scalar, nc.tensor

        consts = ctx.enter_context(tc.tile_pool(name="consts", bufs=1))
        big = ctx.enter_context(tc.tile_pool(name="big", bufs=1))

        # ---- load constants ----
        wih_sb = [consts.tile([128, G], F32R, tag=f"wih{i}", name=f"wih{i}")
                  for i in range(2)]
        for i in range(2):
            nc.sync.dma_start(wih_sb[i][:], d_wih[128 * i:128 * (i + 1), :])
        whh8_sb = consts.tile([128, 2 * 2 * G], F8E4, tag="whh8")
        nc.sync.dma_start(whh8_sb[:], d_whh8[:].bitcast(F8E4))
        whh8_r = whh8_sb[:].rearrange("p (pr ks n) -> p pr ks n", pr=2, ks=2)
        biasrow_sb = consts.tile([1, G], F32R, tag="biasrow")
        nc.sync.dma_start(biasrow_sb[:], d_biasrow[:])
        ones_sb = consts.tile([1, 128], F32R, tag="ones")
        nc.sync.dma_start(ones_sb[:], d_ones[:])
        u1t_sb = consts.tile([128, 4], F32R, tag="u1t")
        nc.sync.dma_start(u1t_sb[:], d_u1t[:])
        u2t_sb = consts.tile([128, 8], F32R, tag="u2t")
        nc.sync.dma_start(u2t_sb[:], d_u2t[:])
        i32s_r = consts.tile([128, 32], F32R, tag="i32s_r")
        nc.sync.dma_start(i32s_r[:], d_i32s[:].bitcast(F32R))
        i32s_f = consts.tile([128, 32], F32, tag="i32s_f")
        nc.sync.dma_start(i32s_f[:], d_i32s[:])
        i128_r = consts.tile([128, 128], F32R, tag="i128_r")
        nc.sync.dma_start(i128_r[:], d_i128[:].bitcast(F32R))
        maskneg_sb = consts.tile([BL, K * t_steps], F32, tag="maskneg")
        nc.sync.dma_start(maskneg_sb[:], d_maskneg[:])
        valid_sb = consts.tile([BL, K], F32, tag="valid")
        nc.sync.dma_start(valid_sb[:], d_valid[:])

        # ---- persistent state ----
        hsT = big.tile([128, t_steps * 128], F32R, tag="hsT")      # [p, t*128+c*32+b]
        c_sb = big.tile([BL, H], F32, tag="c")
        a1ch = big.tile([128, NRC], F32, tag="a1ch")               # a1 by row-chunk
        a2_sb = big.tile([BL, t_steps], F32, tag="a2")

        # ---- loop pools ----
        loop_ctx = ExitStack()
        xt_pool = loop_ctx.enter_context(tc.tile_pool(name="xt", bufs=2))
        xw_pool = loop_ctx.enter_context(tc.tile_pool(name="xw", bufs=2))
        gate_pool = loop_ctx.enter_context(tc.tile_pool(name="gate", bufs=2))
        tmp_pool = loop_ctx.enter_context(tc.tile_pool(name="tmp", bufs=2))
        tct_pool = loop_ctx.enter_context(tc.tile_pool(name="tct", bufs=2))
        h8_pool = loop_ctx.enter_context(tc.tile_pool(name="h8", bufs=2))
        ps_z = loop_ctx.enter_context(tc.tile_pool(name="ps_z", bufs=1, space="PSUM"))
        ps_xw = loop_ctx.enter_context(tc.tile_pool(name="ps_xw", bufs=1, space="PSUM"))
        ps_oc = loop_ctx.enter_context(tc.tile_pool(name="ps_oc", bufs=1, space="PSUM"))
        ps_a1 = loop_ctx.enter_context(tc.tile_pool(name="ps_a1", bufs=1, space="PSUM"))
        ps_a2 = loop_ctx.enter_context(tc.tile_pool(name="ps_a2", bufs=1, space="PSUM"))

        xw_tiles, xtc_tiles, pz_tiles = {}, {}, {}

        def emit_xw_quarter(rr, q):
            """PE matmuls for xw chunk rr, column quarter q -> pending PSUM.

            Returns (pxw, pa1): caller emits the PSUM->SBUF copies at its
            chosen queue positions. pa1 is non-None only at q==3.
            """
            if q == 0:
                xtc = [xt_pool.tile([128, 128], F32R, tag=f"xtc{kd}",
                                    name=f"xtc{kd}_{rr}") for kd in range(2)]
                for kd in range(2):
                    nc.sync.dma_start(
                        xtc[kd][:],
                        d_xT[128 * kd:128 * (kd + 1), 128 * rr:128 * (rr + 1)])
                xtc_tiles[rr] = xtc
                xw_tiles[rr] = xw_pool.tile([128, G], F32R, tag="xw",
                                            name=f"xw{rr}")
            xtc = xtc_tiles[rr]
            pxw = ps_xw.tile([128, 512], F32, tag="pxw")
            for kd in range(2):
                nt.matmul(pxw[:], xtc[kd],
                          wih_sb[kd][:, 512 * q:512 * (q + 1)],
                          start=(kd == 0), stop=False)
            nt.matmul(pxw[:], ones_sb[:], biasrow_sb[:, 512 * q:512 * (q + 1)],
                      start=False, stop=True)
            pa1 = None
            if q == 3:
                pa1 = ps_a1.tile([128, 2], F32, tag="pa1")
                for kd in range(2):
                    nt.matmul(pa1[:], xtc[kd], u1t_sb[:, 2 * kd:2 * kd + 2],
                              start=(kd == 0), stop=(kd == 1))
            return pxw, pa1

        def emit_injects(t):
            """Identity matmuls seeding pz[t] with xw rows (+bias).

            One PSUM tile PER GATE so each bank's accumulation group closes
            independently — readers (per-gate activations) otherwise wait
            for the whole tile's group, serializing the tail after all whh.
            """
            pzs = [ps_z.tile([BL, 512], F32, tag=f"pz{n}", name=f"pz{n}_{t}")
                   for n in range(4)]
            pz_tiles[t] = pzs
            rn, tn4 = divmod(t, 4)
            xwn = xw_tiles[rn]
            last = (t == 0)   # t=0 has no whh accumulation
            for n in range(4):
                nt.matmul(pzs[n][:],
                          i32s_r[32 * tn4:32 * (tn4 + 1), :],
                          xwn[32 * tn4:32 * (tn4 + 1), 512 * n:512 * (n + 1)],
                          start=True, stop=last,
                          tile_position=(32 * tn4, 0))

        # ---- preamble: xw chunk 0, chunk 1 quarter 0, injects for t=0 ----
        for q in range(4):
            pxw, pa1 = emit_xw_quarter(0, q)
            if q % 2 == 0:
                ns.copy(xw_tiles[0][:, 512 * q:512 * (q + 1)], pxw[:])
            else:
                nv.tensor_copy(xw_tiles[0][:, 512 * q:512 * (q + 1)], pxw[:])
            if pa1 is not None:
                ns.copy(a1ch[:, 0:1], pa1[:, 0:1])
        pxw, _ = emit_xw_quarter(1, 0)
        ns.copy(xw_tiles[1][:, 0:512], pxw[:])
        emit_injects(0)

        # gate column order: n0=g, n1=i, n2=f, n3=o (host perm matches)
        for t in range(t_steps):
            r, t4 = divmod(t, 4)
            pzs = pz_tiles.pop(t)
            # --- PE: whh accumulation, fp8 DoubleRow (K=256 per pass),
            # n-outer so gate chunks finish early
            if t > 0:
                h8p = h8_prev[:].rearrange("p (pr ks b) -> p pr ks b",
                                           pr=2, ks=2)
                for n in range(4):
                    for pr in range(2):
                        nt.matmul(
                            pzs[n][:], h8p[:, pr],
                            whh8_r[:, pr, :, 512 * n:512 * (n + 1)],
                            start=False, stop=(pr == 1), perf_mode=DROW)
            # --- PE: a2 for t-1 (stationary = hsT chunks, tiny N)
            pa2 = None
            if t >= 1:
                pa2 = ps_a2.tile([BL, 2], F32, tag="pa2")
                for c4 in range(4):
                    nt.matmul(pa2[:],
                              hsT[:, (t - 1) * 128 + 32 * c4:(t - 1) * 128 + 32 * (c4 + 1)],
                              u2t_sb[:, 2 * c4:2 * c4 + 2],
                              start=(c4 == 0), stop=(c4 == 3))
            # --- PE: xw prefetch, one quarter per iteration, shifted one
            # iteration early (quarter q of chunk rr at t = 4*rr + q - 5) so
            # every quarter's copy sits at a queue tail at least one full
            # iteration before the injects that read it — never on the
            # critical chain. t4==3 iterations carry no xw work.
            xw_pends = []
            xq = (t + 1) % 4
            xrr = (t + 5 - xq) // 4
            if xrr < NRC:
                xw_pends.append((xrr, xq) + emit_xw_quarter(xrr, xq))
            # --- ACT: per-gate activations (z cols [g | i | f | o])
            gg = gate_pool.tile([BL, 512], F32, tag="gg")
            ns.activation(gg[:], pzs[0][:], AFT.Tanh)
            si = gate_pool.tile([BL, 512], F32, tag="si")
            ns.activation(si[:], pzs[1][:], AFT.Sigmoid)
            sf = gate_pool.tile([BL, 512], F32, tag="sf")
            ns.activation(sf[:], pzs[2][:], AFT.Sigmoid)
            so = gate_pool.tile([BL, 512], F32, tag="so")
            ns.activation(so[:], pzs[3][:], AFT.Sigmoid)
            # --- DVE: c update
            if t == 0:
                nv.tensor_tensor(c_sb[:], si[:], gg[:], op=ALU.mult)
            else:
                tig = tmp_pool.tile([BL, H], F32, tag="tig")
                nv.tensor_tensor(tig[:], si[:], gg[:], op=ALU.mult)
                tfc = tmp_pool.tile([BL, H], F32, tag="tfc")
                nv.tensor_tensor(tfc[:], sf[:], c_sb[:], op=ALU.mult)
                nv.tensor_tensor(c_sb[:], tfc[:], tig[:], op=ALU.add)
            # --- PE: transposes of sig_o and c into one PSUM tile
            psOC = ps_oc.tile([128, 256], F32, tag="psOC")
            for c4 in range(4):
                nt.transpose(psOC[:, 32 * c4:32 * (c4 + 1)],
                             so[:, 128 * c4:128 * (c4 + 1)], i32s_f[0:32, :])
            for c4 in range(4):
                nt.transpose(psOC[:, 128 + 32 * c4:128 + 32 * (c4 + 1)],
                             c_sb[:, 128 * c4:128 * (c4 + 1)], i32s_f[0:32, :])
            # --- PE: injects for t+1 (fill the tail stall, keep PE warm)
            if t + 1 < t_steps:
                emit_injects(t + 1)
            # --- ACT: tanh on transposed c (128-partition, short free dim)
            tcT = tct_pool.tile([128, 128], F32, tag="tcT")
            ns.activation(tcT[:], psOC[:, 128:256], AFT.Tanh)
            # --- DVE: h8 = fp8(sig_oT * tanh_cT) halves feed next whh first,
            # then the fp32r hsT copy for pooling/a2 (off the critical path)
            h8 = h8_pool.tile([128, 128], F8E4, tag="h8")
            for hh in range(2):
                nv.tensor_tensor(
                    h8[:, 64 * hh:64 * (hh + 1)],
                    psOC[:, 64 * hh:64 * (hh + 1)],
                    tcT[:, 64 * hh:64 * (hh + 1)], op=ALU.mult)
            h8_prev = h8
            nv.tensor_tensor(hsT[:, t * 128:(t + 1) * 128],
                             psOC[:, 0:128], tcT[:], op=ALU.mult)
            # --- DVE: a2 copy-out
            if pa2 is not None:
                nv.tensor_copy(a2_sb[:, t - 1:t], pa2[:, 0:1])
            # --- queue-tail copies: xw quarters + a1
            for rr, q, pxw_pend, pa1_pend in xw_pends:
                if q % 2 == 0:
                    ns.copy(xw_tiles[rr][:, 512 * q:512 * (q + 1)],
                            pxw_pend[:])
                else:
                    nv.tensor_copy(xw_tiles[rr][:, 512 * q:512 * (q + 1)],
                                   pxw_pend[:])
                if pa1_pend is not None:
                    ns.copy(a1ch[:, rr:rr + 1], pa1_pend[:, 0:1])
            xw_tiles.pop(r - 1, None)
            xtc_tiles.pop(r - 1, None)

        # --- a2 for the final step
        pa2 = ps_a2.tile([BL, 2], F32, tag="pa2")
        for c4 in range(4):
            nt.matmul(pa2[:],
                      hsT[:, (t_steps - 1) * 128 + 32 * c4:(t_steps - 1) * 128 + 32 * (c4 + 1)],
                      u2t_sb[:, 2 * c4:2 * c4 + 2],
                      start=(c4 == 0), stop=(c4 == 3))
        nv.tensor_copy(a2_sb[:, t_steps - 1:t_steps], pa2[:, 0:1])

        loop_ctx.close()

        # ---- post-loop: attention scores + softmax + pooling ----
        post = ctx.enter_context(tc.tile_pool(name="post", bufs=1))
        ps_t = ctx.enter_context(tc.tile_pool(name="ps_t", bufs=2, space="PSUM"))
        ps_pool = ctx.enter_context(tc.tile_pool(name="ps_pool", bufs=4, space="PSUM"))
        stg_pool = ctx.enter_context(tc.tile_pool(name="stg", bufs=4))
        hsb_pool = ctx.enter_context(tc.tile_pool(name="hsb", bufs=2))

        # a1 assembly: a1bp[b, 4r+c] = a1ch[32c+b, r]
        a1bp = post.tile([BL, t_steps], F32, tag="a1bp")
        for c in range(4):
            nv.tensor_copy(a1bp[:].rearrange("b (r c) -> b r c", c=4)[:, :, c],
                           a1ch[32 * c:32 * (c + 1), :])
        abp = post.tile([BL, t_steps], F32, tag="abp")
        nv.tensor_tensor(abp[:], a1bp[:], a2_sb[:], op=ALU.add)
        ns.activation(abp[:], abp[:], AFT.Sigmoid)

        # softmax per slot k -> wT [t, 4b+k] (fp32r for the pooling matmul)
        wT = post.tile([t_steps, K * BL], F32R, tag="wT")
        for k in range(K):
            sc = post.tile([BL, t_steps], F32, tag=f"sc{k}")
            nv.tensor_tensor(sc[:], abp[:],
                             maskneg_sb[:, t_steps * k:t_steps * (k + 1)], op=ALU.add)
            mneg = post.tile([BL, 1], F32, tag=f"mneg{k}")
            nv.tensor_reduce(mneg[:], sc[:], axis=mybir.AxisListType.X,
                             op=ALU.max, negate=True)
            ek = post.tile([BL, t_steps], F32, tag=f"ek{k}")
            sk = post.tile([BL, 1], F32, tag=f"sk{k}")
            ns.activation(ek[:], sc[:], AFT.Exp, bias=mneg[:], accum_out=sk[:])
            rk = post.tile([BL, 1], F32, tag=f"rk{k}")
            nv.reciprocal(rk[:], sk[:])
            wk = post.tile([BL, t_steps], F32, tag=f"wk{k}")
            nv.tensor_scalar(out=wk[:], in0=ek[:], scalar1=rk[:],
                             scalar2=valid_sb[:, k:k + 1], op0=ALU.mult, op1=ALU.mult)
            # transpose into wT columns k::4  (wT[t, 4b+k])
            pwT = ps_t.tile([128, 32], F32, tag="pwT")
            nt.transpose(pwT[0:t_steps, :], wk[:], i32s_f[0:32, :])
            nv.tensor_copy(wT[:].rearrange("t (b k) -> t b k", k=4)[:, :, k],
                           pwT[0:t_steps, :])

        # pooling: per b, rebuild hs_b [t, h] via 4 PE transposes, then [4,T]@[T,H]
        hsT_r = hsT[:].rearrange("p (t c b) -> p t c b", c=4, b=BL)
        for b in range(BL):
            hsb = hsb_pool.tile([t_steps, H], F32R, tag="hsb")
            for c in range(4):
                pt = ps_t.tile([128, 128], F32R, tag="pt")
                nt.transpose(pt[0:t_steps, :], hsT_r[:, :, c, b], i128_r[:])
                if c % 2 == 0:
                    ns.copy(hsb[:, 128 * c:128 * (c + 1)], pt[0:t_steps, :])
                else:
                    nv.tensor_copy(hsb[:, 128 * c:128 * (c + 1)], pt[0:t_steps, :])
            pp = ps_pool.tile([K, H], F32, tag="pp")
            nt.matmul(pp[:], wT[0:t_steps, 4 * b:4 * (b + 1)], hsb[:],
                      start=True, stop=True)
            so = stg_pool.tile([K, H], F32, tag="so")
            ns.copy(so[:], pp[:])
            nc.sync.dma_start(d_out[K * b:K * (b + 1), :], so[:])

    nc.compile()
    return nc


def _host_prep(x, W_ih, W_hh, b_ih, b_hh, A1, A2, v1, lengths, label_len):
    assert int(label_len) == K
    # torch gate rows (i,f,g,o) -> z column order (g,i,f,o)
    perm = np.concatenate([np.arange(1024, 1536), np.arange(0, 512),
                           np.arange(512, 1024), np.arange(1536, 2048)])
    wih = np.ascontiguousarray(W_ih[perm].T, dtype=np.float32)          # [256, 2048]
    whhT = np.asarray(W_hh[perm].T, dtype=np.float32)                   # [512, 2048]
    # fp8 DoubleRow pair layout: whh8[p, pr, ks, n] = WhhT[256*pr+128*ks+p, n]
    whh8 = whhT.reshape(2, 2, 128, G).transpose(2, 0, 1, 3).reshape(128, 4 * G)
    whh8 = np.ascontiguousarray(whh8).astype(ml_dtypes.float8_e4m3).view(np.uint8)
    biasrow = ((b_ih + b_hh)[perm]).astype(np.float32).reshape(1, G)
    u1 = (v1 @ A1)[0].astype(np.float32)                                # [256]
    u2 = (v1 @ A2)[0].astype(np.float32)                                # [512]
    u1t = np.zeros((128, 4), dtype=np.float32)                          # [128, 4]
    u1t[:, 0] = u1[0:128]
    u1t[:, 2] = u1[128:256]
    u2t = np.zeros((128, 8), dtype=np.float32)                          # [128, 8]
    for c in range(4):
        u2t[:, 2 * c] = u2[128 * c:128 * (c + 1)]
    i32s = np.zeros((128, 32), dtype=np.float32)
    i32s[np.arange(128), np.arange(128) % 32] = 1.0
    i128 = np.eye(128, dtype=np.float32)

    shared = dict(wih=wih, whh8=whh8, biasrow=biasrow, u1t=u1t, u2t=u2t,
                  i32s=i32s, i128=i128, onesrow=np.ones((1, 128), dtype=np.float32))

    in_maps = []
    for cidx in range(NC):
        sl = slice(cidx * BL, (cidx + 1) * BL)
        xc = x[:, sl, :]                                                # [T, 32, D]
        xT = np.ascontiguousarray(xc.reshape(T * BL, D).T, dtype=np.float32)
        ln = lengths[sl].astype(np.int64)
        t_start = np.maximum(ln - K, 0)
        t_k = t_start[:, None] + np.arange(K)[None, :]                  # [32, 4]
        valid = (t_k <= (ln[:, None] - 1))                              # [32, 4]
        tt = np.arange(T)
        mask = (tt[None, None, :] <= t_k[:, :, None]) & valid[:, :, None]  # [b, k, t]
        maskneg = np.where(mask, 0.0, NEG_INF).astype(np.float32)
        maskneg = np.ascontiguousarray(maskneg.reshape(BL, K * T))      # k-major cols
        in_maps.append(dict(shared, xT=xT, maskneg=maskneg,
                            valid=valid.astype(np.float32)))
    return in_maps


def kernel(**inputs) -> np.ndarray:
    inputs = {k: np.asarray(v) if not np.isscalar(v) else v for k, v in inputs.items()}
    in_maps = _host_prep(**inputs)
    if "nc" not in _cached:
        _cached["nc"] = _build_program()
    nc = _cached["nc"]
    res = run_bass_kernel_spmd(nc, in_maps, core_ids=list(range(NC)))
    outs = []
    for cidx in range(NC):
        o = res.results[cidx]["out"]                                    # [128, 512]
        outs.append(o.reshape(BL, K, H))
    return np.concatenate(outs, axis=0).astype(np.float32)              # [256, 4, 512]


# revision 26
# speedup vs baseline: 2.0426x; 1.0175x over previous
"""Trainium2 Bass kernel for DUPN-style LSTM + windowed-softmax attention pooling.

Math (per batch element b):
  LSTM over T=128 steps (torch gate order), hidden H=512, input D=256.
  a[t] = sigmoid(x[t]·u1 + h[t]·u2), u1 = (v1@A1)^T, u2 = (v1@A2)^T  (folded)
  out[b,k,:] = softmax-pooled sum of h[t] over window t <= t_k, for 4 slots.

Sharding: data-parallel over batch, 32 per core x 8 cores, weights replicated.

Per-core schedule (v2 — pipelined tail, ~7.5us/step target):
  - z gate-column order is (g, i, f, o) so the c-update chain starts as soon
    as the first whh column chunks finish; per-gate [32,512] activations
    overlap the remaining whh matmuls.
  - h is kept ONLY in transposed form: sig(o) and c are PE-transposed
    separately into one PSUM tile, tanh runs on the 128-partition cT, and a
    DVE multiply writes hsT (fp32r) directly — no scalar-copy on the
    recurrence critical path.
  - xw = x@W_ih^T + bias is computed in per-step quarters ([128,512] PSUM,
    3 matmuls each) so the PE prefetch work is spread evenly; quarter
    copies alternate scalar/vector engines at queue tails.
  - a2[t] = h_t·u2 via 4 tiny stationary-reuse matmuls (N=2) during step
    t+1; a1 via u1 matmuls on the xw input chunks.
  - next-step xw injects (identity matmuls, start=True) are emitted between
    the whh block and the transposes to fill the PE stall while the tail
    chain runs, keeping the PE p-state warm.
  - PSUM: pz 4 banks + pxw 1 + psOC 1 + pa1 1 + pa2 1 = 8.
  - Post-loop: windowed softmax with host-built masks, pooling via per-b
    [4,T]@[T,H] matmuls on PE-transposed hs.
"""
import sys

if "/opt/trn_rl_repo" not in sys.path:
    sys.path.insert(0, "/opt/trn_rl_repo")

import numpy as np
import ml_dtypes
import concourse.bass as bass
import concourse.bacc as bacc
import concourse.tile as tile
from concourse import mybir
from concourse.bass_utils import run_bass_kernel_spmd
from contextlib import ExitStack

F32 = mybir.dt.float32
F32R = mybir.dt.float32r
F8E4 = mybir.dt.float8e4
BF16 = mybir.dt.bfloat16
U8 = mybir.dt.uint8
AFT = mybir.ActivationFunctionType
ALU = mybir.AluOpType
DROW = mybir.MatmulPerfMode.DoubleRow

T, BF, D, H, K, NC = 128, 256, 256, 512, 4, 8
BL = BF // NC          # 32 batch per core
G = 4 * H              # 2048
NEG_INF = -1e9

_cached = {}


def _build_program(t_steps=T):
    nc = bacc.Bacc()
    # ---- DRAM I/O (fp32r where feeding matmuls; same bytes as fp32) ----
    d_xT = nc.declare_dram_parameter("xT", [D, t_steps * BL], F32R, isOutput=False)
    d_wih = nc.declare_dram_parameter("wih", [D, G], F32R, isOutput=False)
    # W_hh^T in fp8e4, DoubleRow pair layout [p, pair, ksub, n] flattened
    d_whh8 = nc.declare_dram_parameter("whh8", [128, 2 * 2 * G], U8, isOutput=False)
    d_biasrow = nc.declare_dram_parameter("biasrow", [1, G], F32R, isOutput=False)
    d_ones = nc.declare_dram_parameter("onesrow", [1, 128], F32R, isOutput=False)
    d_u1t = nc.declare_dram_parameter("u1t", [128, 2 * (D // 128)], F32R, isOutput=False)
    d_u2t = nc.declare_dram_parameter("u2t", [128, 2 * (H // 128)], F32R, isOutput=False)
    d_i32s = nc.declare_dram_parameter("i32s", [128, 32], F32, isOutput=False)
    d_i128 = nc.declare_dram_parameter("i128", [128, 128], F32, isOutput=False)
    d_maskneg = nc.declare_dram_parameter("maskneg", [BL, K * t_steps], F32, isOutput=False)
    d_valid = nc.declare_dram_parameter("valid", [BL, K], F32, isOutput=False)
    d_out = nc.declare_dram_parameter("out", [BL * K, H], F32, isOutput=True)

    NRC = t_steps // 4     # row chunks of 128 rows (4 timesteps each)

    with tile.TileContext(nc) as tc, ExitStack() as ctx:
        nv, ns, nt = nc.vector, nc.scalar, nc.tensor

        consts = ctx.enter_context(tc.tile_pool(name="consts", bufs=1))
        big = ctx.enter_context(tc.tile_pool(name="big", bufs=1))

        # ---- load constants ----
        wih_sb = [consts.tile([128, G], F32R, tag=f"wih{i}", name=f"wih{i}")
                  for i in range(2)]
        for i in range(2):
            nc.sync.dma_start(wih_sb[i][:], d_wih[128 * i:128 * (i + 1), :])
        whh8_sb = consts.tile([128, 2 * 2 * G], F8E4, tag="whh8")
        nc.sync.dma_start(whh8_sb[:], d_whh8[:].bitcast(F8E4))
        whh8_r = whh8_sb[:].rearrange("p (pr ks n) -> p pr ks n", pr=2, ks=2)
        biasrow_sb = consts.tile([1, G], F32R, tag="biasrow")
        nc.sync.dma_start(biasrow_sb[:], d_biasrow[:])
        ones_sb = consts.tile([1, 128], F32R, tag="ones")
        nc.sync.dma_start(ones_sb[:], d_ones[:])
        u1t_sb = consts.tile([128, 4], F32R, tag="u1t")
        nc.sync.dma_start(u1t_sb[:], d_u1t[:])
        u2t_sb = consts.tile([128, 8], F32R, tag="u2t")
        nc.sync.dma_start(u2t_sb[:], d_u2t[:])
        i32s_r = consts.tile([128, 32], F32R, tag="i32s_r")
        nc.sync.dma_start(i32s_r[:], d_i32s[:].bitcast(F32R))
        i32s_f = consts.tile([128, 32], F32, tag="i32s_f")
        nc.sync.dma_start(i32s_f[:], d_i32s[:])
        i32s_bf = consts.tile([128, 32], BF16, tag="i32s_bf")
        ns.copy(i32s_bf[:], i32s_f[:])
        i128_r = consts.tile([128, 128], F32R, tag="i128_r")
        nc.sync.dma_start(i128_r[:], d_i128[:].bitcast(F32R))
        maskneg_sb = consts.tile([BL, K * t_steps], F32, tag="maskneg")
        nc.sync.dma_start(maskneg_sb[:], d_maskneg[:])
        valid_sb = consts.tile([BL, K], F32, tag="valid")
        nc.sync.dma_start(valid_sb[:], d_valid[:])

        # ---- persistent state ----
        hsT = big.tile([128, t_steps * 128], F32R, tag="hsT")      # [p, t*128+c*32+b]
        c_sb = big.tile([BL, H], BF16, tag="c")
        a1ch = big.tile([128, NRC], F32, tag="a1ch")               # a1 by row-chunk
        a2_sb = big.tile([BL, t_steps], F32, tag="a2")

        # ---- loop pools ----
        loop_ctx = ExitStack()
        xt_pool = loop_ctx.enter_context(tc.tile_pool(name="xt", bufs=2))
        xw_pool = loop_ctx.enter_context(tc.tile_pool(name="xw", bufs=2))
        gate_pool = loop_ctx.enter_context(tc.tile_pool(name="gate", bufs=2))
        tmp_pool = loop_ctx.enter_context(tc.tile_pool(name="tmp", bufs=2))
        tct_pool = loop_ctx.enter_context(tc.tile_pool(name="tct", bufs=2))
        h8_pool = loop_ctx.enter_context(tc.tile_pool(name="h8", bufs=2))
        ps_z = loop_ctx.enter_context(tc.tile_pool(name="ps_z", bufs=1, space="PSUM"))
        ps_xw = loop_ctx.enter_context(tc.tile_pool(name="ps_xw", bufs=1, space="PSUM"))
        ps_oc = loop_ctx.enter_context(tc.tile_pool(name="ps_oc", bufs=1, space="PSUM"))
        ps_a1 = loop_ctx.enter_context(tc.tile_pool(name="ps_a1", bufs=1, space="PSUM"))
        ps_a2 = loop_ctx.enter_context(tc.tile_pool(name="ps_a2", bufs=1, space="PSUM"))

        xw_tiles, xtc_tiles, pz_tiles = {}, {}, {}

        def emit_xw_quarter(rr, q):
            """PE matmuls for xw chunk rr, column quarter q -> pending PSUM.

            Returns (pxw, pa1): caller emits the PSUM->SBUF copies at its
            chosen queue positions. pa1 is non-None only at q==3.
            """
            if q == 0:
                xtc = [xt_pool.tile([128, 128], F32R, tag=f"xtc{kd}",
                                    name=f"xtc{kd}_{rr}") for kd in range(2)]
                for kd in range(2):
                    nc.sync.dma_start(
                        xtc[kd][:],
                        d_xT[128 * kd:128 * (kd + 1), 128 * rr:128 * (rr + 1)])
                xtc_tiles[rr] = xtc
                xw_tiles[rr] = xw_pool.tile([128, G], F32R, tag="xw",
                                            name=f"xw{rr}")
            xtc = xtc_tiles[rr]
            pxw = ps_xw.tile([128, 512], F32, tag="pxw")
            for kd in range(2):
                nt.matmul(pxw[:], xtc[kd],
                          wih_sb[kd][:, 512 * q:512 * (q + 1)],
                          start=(kd == 0), stop=False)
            nt.matmul(pxw[:], ones_sb[:], biasrow_sb[:, 512 * q:512 * (q + 1)],
                      start=False, stop=True)
            pa1 = None
            if q == 3:
                pa1 = ps_a1.tile([128, 2], F32, tag="pa1")
                for kd in range(2):
                    nt.matmul(pa1[:], xtc[kd], u1t_sb[:, 2 * kd:2 * kd + 2],
                              start=(kd == 0), stop=(kd == 1))
            return pxw, pa1

        def emit_injects(t):
            """Identity matmuls seeding pz[t] with xw rows (+bias).

            One PSUM tile PER GATE so each bank's accumulation group closes
            independently — readers (per-gate activations) otherwise wait
            for the whole tile's group, serializing the tail after all whh.
            """
            pzs = [ps_z.tile([BL, 512], F32, tag=f"pz{n}", name=f"pz{n}_{t}")
                   for n in range(4)]
            pz_tiles[t] = pzs
            rn, tn4 = divmod(t, 4)
            xwn = xw_tiles[rn]
            last = (t == 0)   # t=0 has no whh accumulation
            for n in range(4):
                nt.matmul(pzs[n][:],
                          i32s_r[32 * tn4:32 * (tn4 + 1), :],
                          xwn[32 * tn4:32 * (tn4 + 1), 512 * n:512 * (n + 1)],
                          start=True, stop=last,
                          tile_position=(32 * tn4, 0))

        # ---- preamble: xw chunk 0, chunk 1 quarter 0, injects for t=0 ----
        for q in range(4):
            pxw, pa1 = emit_xw_quarter(0, q)
            if q % 2 == 0:
                ns.copy(xw_tiles[0][:, 512 * q:512 * (q + 1)], pxw[:])
            else:
                nv.tensor_copy(xw_tiles[0][:, 512 * q:512 * (q + 1)], pxw[:])
            if pa1 is not None:
                ns.copy(a1ch[:, 0:1], pa1[:, 0:1])
        pxw, _ = emit_xw_quarter(1, 0)
        ns.copy(xw_tiles[1][:, 0:512], pxw[:])
        emit_injects(0)

        # gate column order: n0=g, n1=i, n2=f, n3=o (host perm matches)
        for t in range(t_steps):
            r, t4 = divmod(t, 4)
            pzs = pz_tiles.pop(t)
            # --- PE: whh accumulation, fp8 DoubleRow (K=256 per pass),
            # n-outer so gate chunks finish early
            if t > 0:
                h8p = h8_prev[:].rearrange("p (pr ks b) -> p pr ks b",
                                           pr=2, ks=2)
                for n in range(4):
                    for pr in range(2):
                        nt.matmul(
                            pzs[n][:], h8p[:, pr],
                            whh8_r[:, pr, :, 512 * n:512 * (n + 1)],
                            start=False, stop=(pr == 1), perf_mode=DROW)
            # --- PE: a2 for t-1 (stationary = hsT chunks, tiny N)
            pa2 = None
            if t >= 1:
                pa2 = ps_a2.tile([BL, 2], F32, tag="pa2")
                for c4 in range(4):
                    nt.matmul(pa2[:],
                              hsT[:, (t - 1) * 128 + 32 * c4:(t - 1) * 128 + 32 * (c4 + 1)],
                              u2t_sb[:, 2 * c4:2 * c4 + 2],
                              start=(c4 == 0), stop=(c4 == 3))
            # --- PE: xw prefetch, one quarter per iteration, shifted one
            # iteration early (quarter q of chunk rr at t = 4*rr + q - 5) so
            # every quarter's copy sits at a queue tail at least one full
            # iteration before the injects that read it — never on the
            # critical chain. t4==3 iterations carry no xw work.
            xw_pends = []
            xq = (t + 1) % 4
            xrr = (t + 5 - xq) // 4
            if xrr < NRC:
                xw_pends.append((xrr, xq) + emit_xw_quarter(xrr, xq))
            # --- ACT: per-gate activations (z cols [g | i | f | o])
            gg = gate_pool.tile([BL, 512], BF16, tag="gg")
            ns.activation(gg[:], pzs[0][:], AFT.Tanh)
            si = gate_pool.tile([BL, 512], BF16, tag="si")
            ns.activation(si[:], pzs[1][:], AFT.Sigmoid)
            sf = gate_pool.tile([BL, 512], BF16, tag="sf")
            ns.activation(sf[:], pzs[2][:], AFT.Sigmoid)
            so = gate_pool.tile([BL, 512], BF16, tag="so")
            ns.activation(so[:], pzs[3][:], AFT.Sigmoid)
            # --- DVE: c update
            if t == 0:
                nv.tensor_tensor(c_sb[:], si[:], gg[:], op=ALU.mult)
            else:
                tig = tmp_pool.tile([BL, H], BF16, tag="tig")
                nv.tensor_tensor(tig[:], si[:], gg[:], op=ALU.mult)
                tfc = tmp_pool.tile([BL, H], BF16, tag="tfc")
                nv.tensor_tensor(tfc[:], sf[:], c_sb[:], op=ALU.mult)
                nv.tensor_tensor(c_sb[:], tfc[:], tig[:], op=ALU.add)
            # --- PE: transposes of sig_o and c into one PSUM tile
            psOC = ps_oc.tile([128, 256], BF16, tag="psOC")
            for c4 in range(4):
                nt.transpose(psOC[:, 32 * c4:32 * (c4 + 1)],
                             so[:, 128 * c4:128 * (c4 + 1)], i32s_bf[0:32, :])
            for c4 in range(4):
                nt.transpose(psOC[:, 128 + 32 * c4:128 + 32 * (c4 + 1)],
                             c_sb[:, 128 * c4:128 * (c4 + 1)], i32s_bf[0:32, :])
            # --- PE: injects for t+1 (fill the tail stall, keep PE warm)
            if t + 1 < t_steps:
                emit_injects(t + 1)
            # --- ACT: tanh on transposed c (128-partition, short free dim)
            tcT = tct_pool.tile([128, 128], BF16, tag="tcT")
            ns.activation(tcT[:], psOC[:, 128:256], AFT.Tanh)
            # --- DVE: h8 = fp8(sig_oT * tanh_cT) halves feed next whh first,
            # then the fp32r hsT copy for pooling/a2 (off the critical path)
            h8 = h8_pool.tile([128, 128], F8E4, tag="h8")
            nv.tensor_tensor(h8[:], psOC[:, 0:128], tcT[:], op=ALU.mult)
            h8_prev = h8
            nv.tensor_tensor(hsT[:, t * 128:(t + 1) * 128],
                             psOC[:, 0:128], tcT[:], op=ALU.mult)
            # --- DVE: a2 copy-out
            if pa2 is not None:
                nv.tensor_copy(a2_sb[:, t - 1:t], pa2[:, 0:1])
            # --- queue-tail copies: xw quarters + a1
            for rr, q, pxw_pend, pa1_pend in xw_pends:
                if q % 2 == 0:
                    ns.copy(xw_tiles[rr][:, 512 * q:512 * (q + 1)],
                            pxw_pend[:])
                else:
                    nv.tensor_copy(xw_tiles[rr][:, 512 * q:512 * (q + 1)],
                                   pxw_pend[:])
                if pa1_pend is not None:
                    ns.copy(a1ch[:, rr:rr + 1], pa1_pend[:, 0:1])
            xw_tiles.pop(r - 1, None)
            xtc_tiles.pop(r - 1, None)

        # --- a2 for the final step
        pa2 = ps_a2.tile([BL, 2], F32, tag="pa2")
        for c4 in range(4):
            nt.matmul(pa2[:],
                      hsT[:, (t_steps - 1) * 128 + 32 * c4:(t_steps - 1) * 128 + 32 * (c4 + 1)],
                      u2t_sb[:, 2 * c4:2 * c4 + 2],
                      start=(c4 == 0), stop=(c4 == 3))
        nv.tensor_copy(a2_sb[:, t_steps - 1:t_steps], pa2[:, 0:1])

        loop_ctx.close()

        # ---- post-loop: attention scores + softmax + pooling ----
        post = ctx.enter_context(tc.tile_pool(name="post", bufs=1))
        ps_t = ctx.enter_context(tc.tile_pool(name="ps_t", bufs=2, space="PSUM"))
        ps_pool = ctx.enter_context(tc.tile_pool(name="ps_pool", bufs=4, space="PSUM"))
        stg_pool = ctx.enter_context(tc.tile_pool(name="stg", bufs=4))
        hsb_pool = ctx.enter_context(tc.tile_pool(name="hsb", bufs=2))

        # a1 assembly: a1bp[b, 4r+c] = a1ch[32c+b, r]
        a1bp = post.tile([BL, t_steps], F32, tag="a1bp")
        for c in range(4):
            nv.tensor_copy(a1bp[:].rearrange("b (r c) -> b r c", c=4)[:, :, c],
                           a1ch[32 * c:32 * (c + 1), :])
        abp = post.tile([BL, t_steps], F32, tag="abp")
        nv.tensor_tensor(abp[:], a1bp[:], a2_sb[:], op=ALU.add)
        ns.activation(abp[:], abp[:], AFT.Sigmoid)

        # softmax per slot k -> wT [t, 4b+k] (fp32r for the pooling matmul)
        wT = post.tile([t_steps, K * BL], F32R, tag="wT")
        for k in range(K):
            sc = post.tile([BL, t_steps], F32, tag=f"sc{k}")
            nv.tensor_tensor(sc[:], abp[:],
                             maskneg_sb[:, t_steps * k:t_steps * (k + 1)], op=ALU.add)
            mneg = post.tile([BL, 1], F32, tag=f"mneg{k}")
            nv.tensor_reduce(mneg[:], sc[:], axis=mybir.AxisListType.X,
                             op=ALU.max, negate=True)
            ek = post.tile([BL, t_steps], F32, tag=f"ek{k}")
            sk = post.tile([BL, 1], F32, tag=f"sk{k}")
            ns.activation(ek[:], sc[:], AFT.Exp, bias=mneg[:], accum_out=sk[:])
            rk = post.tile([BL, 1], F32, tag=f"rk{k}")
            nv.reciprocal(rk[:], sk[:])
            wk = post.tile([BL, t_steps], F32, tag=f"wk{k}")
            nv.tensor_scalar(out=wk[:], in0=ek[:], scalar1=rk[:],
                             scalar2=valid_sb[:, k:k + 1], op0=ALU.mult, op1=ALU.mult)
            # transpose into wT columns k::4  (wT[t, 4b+k])
            pwT = ps_t.tile([128, 32], F32, tag="pwT")
            nt.transpose(pwT[0:t_steps, :], wk[:], i32s_f[0:32, :])
            nv.tensor_copy(wT[:].rearrange("t (b k) -> t b k", k=4)[:, :, k],
                           pwT[0:t_steps, :])

        # pooling: per b, rebuild hs_b [t, h] via 4 PE transposes, then [4,T]@[T,H]
        hsT_r = hsT[:].rearrange("p (t c b) -> p t c b", c=4, b=BL)
        for b in range(BL):
            hsb = hsb_pool.tile([t_steps, H], F32R, tag="hsb")
            for c in range(4):
                pt = ps_t.tile([128, 128], F32R, tag="pt")
                nt.transpose(pt[0:t_steps, :], hsT_r[:, :, c, b], i128_r[:])
                if c % 2 == 0:
                    ns.copy(hsb[:, 128 * c:128 * (c + 1)], pt[0:t_steps, :])
                else:
                    nv.tensor_copy(hsb[:, 128 * c:128 * (c + 1)], pt[0:t_steps, :])
            pp = ps_pool.tile([K, H], F32, tag="pp")
            nt.matmul(pp[:], wT[0:t_steps, 4 * b:4 * (b + 1)], hsb[:],
                      start=True, stop=True)
            so = stg_pool.tile([K, H], F32, tag="so")
            ns.copy(so[:], pp[:])
            nc.sync.dma_start(d_out[K * b:K * (b + 1), :], so[:])

    nc.compile()
    return nc


def _host_prep(x, W_ih, W_hh, b_ih, b_hh, A1, A2, v1, lengths, label_len):
    assert int(label_len) == K
    # torch gate rows (i,f,g,o) -> z column order (g,i,f,o)
    perm = np.concatenate([np.arange(1024, 1536), np.arange(0, 512),
                           np.arange(512, 1024), np.arange(1536, 2048)])
    wih = np.ascontiguousarray(W_ih[perm].T, dtype=np.float32)          # [256, 2048]
    whhT = np.asarray(W_hh[perm].T, dtype=np.float32)                   # [512, 2048]
    # fp8 DoubleRow pair layout: whh8[p, pr, ks, n] = WhhT[256*pr+128*ks+p, n]
    whh8 = whhT.reshape(2, 2, 128, G).transpose(2, 0, 1, 3).reshape(128, 4 * G)
    whh8 = np.ascontiguousarray(whh8).astype(ml_dtypes.float8_e4m3).view(np.uint8)
    biasrow = ((b_ih + b_hh)[perm]).astype(np.float32).reshape(1, G)
    u1 = (v1 @ A1)[0].astype(np.float32)                                # [256]
    u2 = (v1 @ A2)[0].astype(np.float32)                                # [512]
    u1t = np.zeros((128, 4), dtype=np.float32)                          # [128, 4]
    u1t[:, 0] = u1[0:128]
    u1t[:, 2] = u1[128:256]
    u2t = np.zeros((128, 8), dtype=np.float32)                          # [128, 8]
    for c in range(4):
        u2t[:, 2 * c] = u2[128 * c:128 * (c + 1)]
    i32s = np.zeros((128, 32), dtype=np.float32)
    i32s[np.arange(128), np.arange(128) % 32] = 1.0
    i128 = np.eye(128, dtype=np.float32)

    shared = dict(wih=wih, whh8=whh8, biasrow=biasrow, u1t=u1t, u2t=u2t,
                  i32s=i32s, i128=i128, onesrow=np.ones((1, 128), dtype=np.float32))

    in_maps = []
    for cidx in range(NC):
        sl = slice(cidx * BL, (cidx + 1) * BL)
        xc = x[:, sl, :]                                                # [T, 32, D]
        xT = np.ascontiguousarray(xc.reshape(T * BL, D).T, dtype=np.float32)
        ln = lengths[sl].astype(np.int64)
        t_start = np.maximum(ln - K, 0)
        t_k = t_start[:, None] + np.arange(K)[None, :]                  # [32, 4]
        valid = (t_k <= (ln[:, None] - 1))                              # [32, 4]
        tt = np.arange(T)
        mask = (tt[None, None, :] <= t_k[:, :, None]) & valid[:, :, None]  # [b, k, t]
        maskneg = np.where(mask, 0.0, NEG_INF).astype(np.float32)
        maskneg = np.ascontiguousarray(maskneg.reshape(BL, K * T))      # k-major cols
        in_maps.append(dict(shared, xT=xT, maskneg=maskneg,
                            valid=valid.astype(np.float32)))
    return in_maps


def kernel(**inputs) -> np.ndarray:
    inputs = {k: np.asarray(v) if not np.isscalar(v) else v for k, v in inputs.items()}
    in_maps = _host_prep(**inputs)
    if "nc" not in _cached:
        _cached["nc"] = _build_program()
    nc = _cached["nc"]
    res = run_bass_kernel_spmd(nc, in_maps, core_ids=list(range(NC)))
    outs = []
    for cidx in range(NC):
        o = res.results[cidx]["out"]                                    # [128, 512]
        outs.append(o.reshape(BL, K, H))
    return np.concatenate(outs, axis=0).astype(np.float32)              # [256, 4, 512]


# revision 27
# speedup vs baseline: 2.0567x; 1.0069x over previous
"""Trainium2 Bass kernel for DUPN-style LSTM + windowed-softmax attention pooling.

Math (per batch element b):
  LSTM over T=128 steps (torch gate order), hidden H=512, input D=256.
  a[t] = sigmoid(x[t]·u1 + h[t]·u2), u1 = (v1@A1)^T, u2 = (v1@A2)^T  (folded)
  out[b,k,:] = softmax-pooled sum of h[t] over window t <= t_k, for 4 slots.

Sharding: data-parallel over batch, 32 per core x 8 cores, weights replicated.

Per-core schedule (v2 — pipelined tail, ~7.5us/step target):
  - z gate-column order is (g, i, f, o) so the c-update chain starts as soon
    as the first whh column chunks finish; per-gate [32,512] activations
    overlap the remaining whh matmuls.
  - h is kept ONLY in transposed form: sig(o) and c are PE-transposed
    separately into one PSUM tile, tanh runs on the 128-partition cT, and a
    DVE multiply writes hsT (fp32r) directly — no scalar-copy on the
    recurrence critical path.
  - xw = x@W_ih^T + bias is computed in per-step quarters ([128,512] PSUM,
    3 matmuls each) so the PE prefetch work is spread evenly; quarter
    copies alternate scalar/vector engines at queue tails.
  - a2[t] = h_t·u2 via 4 tiny stationary-reuse matmuls (N=2) during step
    t+1; a1 via u1 matmuls on the xw input chunks.
  - next-step xw injects (identity matmuls, start=True) are emitted between
    the whh block and the transposes to fill the PE stall while the tail
    chain runs, keeping the PE p-state warm.
  - PSUM: pz 4 banks + pxw 1 + psOC 1 + pa1 1 + pa2 1 = 8.
  - Post-loop: windowed softmax with host-built masks, pooling via per-b
    [4,T]@[T,H] matmuls on PE-transposed hs.
"""
import sys

if "/opt/trn_rl_repo" not in sys.path:
    sys.path.insert(0, "/opt/trn_rl_repo")

import numpy as np
import ml_dtypes
import concourse.bass as bass
import concourse.bacc as bacc
import concourse.tile as tile
from concourse import mybir
from concourse.bass_utils import run_bass_kernel_spmd
from contextlib import ExitStack

F32 = mybir.dt.float32
F32R = mybir.dt.float32r
F8E4 = mybir.dt.float8e4
BF16 = mybir.dt.bfloat16
U8 = mybir.dt.uint8
AFT = mybir.ActivationFunctionType
ALU = mybir.AluOpType
DROW = mybir.MatmulPerfMode.DoubleRow

T, BF, D, H, K, NC = 128, 256, 256, 512, 4, 8
BL = BF // NC          # 32 batch per core
G = 4 * H              # 2048
NEG_INF = -1e9

_cached = {}


def _build_program(t_steps=T):
    nc = bacc.Bacc()
    # ---- DRAM I/O (fp32r where feeding matmuls; same bytes as fp32) ----
    d_xT = nc.declare_dram_parameter("xT", [D, t_steps * BL], F32R, isOutput=False)
    d_wih = nc.declare_dram_parameter("wih", [D, G], F32R, isOutput=False)
    # W_hh^T in fp8e4, DoubleRow pair layout [p, pair, ksub, n] flattened
    d_whh8 = nc.declare_dram_parameter("whh8", [128, 2 * 2 * G], U8, isOutput=False)
    d_biasrow = nc.declare_dram_parameter("biasrow", [1, G], F32R, isOutput=False)
    d_ones = nc.declare_dram_parameter("onesrow", [1, 128], F32R, isOutput=False)
    d_u1t = nc.declare_dram_parameter("u1t", [128, 2 * (D // 128)], F32R, isOutput=False)
    d_u2t = nc.declare_dram_parameter("u2t", [128, 2 * (H // 128)], F32R, isOutput=False)
    d_i32s = nc.declare_dram_parameter("i32s", [128, 32], F32, isOutput=False)
    d_i128 = nc.declare_dram_parameter("i128", [128, 128], F32, isOutput=False)
    d_maskneg = nc.declare_dram_parameter("maskneg", [BL, K * t_steps], F32, isOutput=False)
    d_valid = nc.declare_dram_parameter("valid", [BL, K], F32, isOutput=False)
    d_out = nc.declare_dram_parameter("out", [BL * K, H], F32, isOutput=True)

    NRC = t_steps // 4     # row chunks of 128 rows (4 timesteps each)

    with tile.TileContext(nc) as tc, ExitStack() as ctx:
        nv, ns, nt = nc.vector, nc.scalar, nc.tensor

        consts = ctx.enter_context(tc.tile_pool(name="consts", bufs=1))
        big = ctx.enter_context(tc.tile_pool(name="big", bufs=1))

        # ---- load constants ----
        wih_sb = [consts.tile([128, G], F32R, tag=f"wih{i}", name=f"wih{i}")
                  for i in range(2)]
        for i in range(2):
            nc.sync.dma_start(wih_sb[i][:], d_wih[128 * i:128 * (i + 1), :])
        whh8_sb = consts.tile([128, 2 * 2 * G], F8E4, tag="whh8")
        nc.sync.dma_start(whh8_sb[:], d_whh8[:].bitcast(F8E4))
        whh8_r = whh8_sb[:].rearrange("p (pr ks n) -> p pr ks n", pr=2, ks=2)
        biasrow_sb = consts.tile([1, G], F32R, tag="biasrow")
        nc.sync.dma_start(biasrow_sb[:], d_biasrow[:])
        ones_sb = consts.tile([1, 128], F32R, tag="ones")
        nc.sync.dma_start(ones_sb[:], d_ones[:])
        u1t_sb = consts.tile([128, 4], F32R, tag="u1t")
        nc.sync.dma_start(u1t_sb[:], d_u1t[:])
        u2t_sb = consts.tile([128, 8], F32R, tag="u2t")
        nc.sync.dma_start(u2t_sb[:], d_u2t[:])
        i32s_r = consts.tile([128, 32], F32R, tag="i32s_r")
        nc.sync.dma_start(i32s_r[:], d_i32s[:].bitcast(F32R))
        i32s_f = consts.tile([128, 32], F32, tag="i32s_f")
        nc.sync.dma_start(i32s_f[:], d_i32s[:])
        i32s_bf = consts.tile([128, 32], BF16, tag="i32s_bf")
        ns.copy(i32s_bf[:], i32s_f[:])
        i128_r = consts.tile([128, 128], F32R, tag="i128_r")
        nc.sync.dma_start(i128_r[:], d_i128[:].bitcast(F32R))
        maskneg_sb = consts.tile([BL, K * t_steps], F32, tag="maskneg")
        nc.sync.dma_start(maskneg_sb[:], d_maskneg[:])
        valid_sb = consts.tile([BL, K], F32, tag="valid")
        nc.sync.dma_start(valid_sb[:], d_valid[:])

        # ---- persistent state ----
        hsT = big.tile([128, t_steps * 128], F32R, tag="hsT")      # [p, t*128+c*32+b]
        c_sb = big.tile([BL, H], BF16, tag="c")
        a1ch = big.tile([128, NRC], F32, tag="a1ch")               # a1 by row-chunk
        a2_sb = big.tile([BL, t_steps], F32, tag="a2")

        # ---- loop pools ----
        loop_ctx = ExitStack()
        xt_pool = loop_ctx.enter_context(tc.tile_pool(name="xt", bufs=2))
        xw_pool = loop_ctx.enter_context(tc.tile_pool(name="xw", bufs=2))
        gate_pool = loop_ctx.enter_context(tc.tile_pool(name="gate", bufs=2))
        tmp_pool = loop_ctx.enter_context(tc.tile_pool(name="tmp", bufs=2))
        tct_pool = loop_ctx.enter_context(tc.tile_pool(name="tct", bufs=2))
        h8_pool = loop_ctx.enter_context(tc.tile_pool(name="h8", bufs=2))
        ps_z = loop_ctx.enter_context(tc.tile_pool(name="ps_z", bufs=1, space="PSUM"))
        ps_xw = loop_ctx.enter_context(tc.tile_pool(name="ps_xw", bufs=1, space="PSUM"))
        ps_oc = loop_ctx.enter_context(tc.tile_pool(name="ps_oc", bufs=1, space="PSUM"))
        ps_a1 = loop_ctx.enter_context(tc.tile_pool(name="ps_a1", bufs=1, space="PSUM"))

        xw_tiles, xtc_tiles, pz_tiles = {}, {}, {}

        def emit_xw_quarter(rr, q):
            """PE matmuls for xw chunk rr, column quarter q -> pending PSUM.

            Returns (pxw, pa1): caller emits the PSUM->SBUF copies at its
            chosen queue positions. pa1 is non-None only at q==3.
            """
            if q == 0:
                xtc = [xt_pool.tile([128, 128], F32R, tag=f"xtc{kd}",
                                    name=f"xtc{kd}_{rr}") for kd in range(2)]
                for kd in range(2):
                    nc.sync.dma_start(
                        xtc[kd][:],
                        d_xT[128 * kd:128 * (kd + 1), 128 * rr:128 * (rr + 1)])
                xtc_tiles[rr] = xtc
                xw_tiles[rr] = xw_pool.tile([128, G], F32R, tag="xw",
                                            name=f"xw{rr}")
            xtc = xtc_tiles[rr]
            pxw = ps_xw.tile([128, 512], F32, tag="pxw")
            for kd in range(2):
                nt.matmul(pxw[:], xtc[kd],
                          wih_sb[kd][:, 512 * q:512 * (q + 1)],
                          start=(kd == 0), stop=False)
            nt.matmul(pxw[:], ones_sb[:], biasrow_sb[:, 512 * q:512 * (q + 1)],
                      start=False, stop=True)
            pa1 = None
            if q == 3:
                pa1 = ps_a1.tile([128, 2], F32, tag="pa1")
                for kd in range(2):
                    nt.matmul(pa1[:], xtc[kd], u1t_sb[:, 2 * kd:2 * kd + 2],
                              start=(kd == 0), stop=(kd == 1))
            return pxw, pa1

        def emit_injects(t):
            """Identity matmuls seeding pz[t] with xw rows (+bias).

            One PSUM tile PER GATE so each bank's accumulation group closes
            independently — readers (per-gate activations) otherwise wait
            for the whole tile's group, serializing the tail after all whh.
            """
            pzs = [ps_z.tile([BL, 512], F32, tag=f"pz{n}", name=f"pz{n}_{t}")
                   for n in range(4)]
            pz_tiles[t] = pzs
            rn, tn4 = divmod(t, 4)
            xwn = xw_tiles[rn]
            last = (t == 0)   # t=0 has no whh accumulation
            for n in range(4):
                nt.matmul(pzs[n][:],
                          i32s_r[32 * tn4:32 * (tn4 + 1), :],
                          xwn[32 * tn4:32 * (tn4 + 1), 512 * n:512 * (n + 1)],
                          start=True, stop=last,
                          tile_position=(32 * tn4, 0))

        # ---- preamble: xw chunk 0, chunk 1 quarter 0, injects for t=0 ----
        for q in range(4):
            pxw, pa1 = emit_xw_quarter(0, q)
            if q % 2 == 0:
                ns.copy(xw_tiles[0][:, 512 * q:512 * (q + 1)], pxw[:])
            else:
                nv.tensor_copy(xw_tiles[0][:, 512 * q:512 * (q + 1)], pxw[:])
            if pa1 is not None:
                ns.copy(a1ch[:, 0:1], pa1[:, 0:1])
        pxw, _ = emit_xw_quarter(1, 0)
        ns.copy(xw_tiles[1][:, 0:512], pxw[:])
        emit_injects(0)

        # gate column order: n0=g, n1=i, n2=f, n3=o (host perm matches)
        for t in range(t_steps):
            r, t4 = divmod(t, 4)
            pzs = pz_tiles.pop(t)
            # --- PE: whh accumulation, fp8 DoubleRow (K=256 per pass),
            # n-outer so gate chunks finish early
            if t > 0:
                h8p = h8_prev[:].rearrange("p (pr ks b) -> p pr ks b",
                                           pr=2, ks=2)
                for n in range(4):
                    for pr in range(2):
                        nt.matmul(
                            pzs[n][:], h8p[:, pr],
                            whh8_r[:, pr, :, 512 * n:512 * (n + 1)],
                            start=False, stop=(pr == 1), perf_mode=DROW)
            # --- PE: xw prefetch, one quarter per iteration, shifted one
            # iteration early (quarter q of chunk rr at t = 4*rr + q - 5) so
            # every quarter's copy sits at a queue tail at least one full
            # iteration before the injects that read it — never on the
            # critical chain. t4==3 iterations carry no xw work.
            xw_pends = []
            xq = (t + 1) % 4
            xrr = (t + 5 - xq) // 4
            if xrr < NRC:
                xw_pends.append((xrr, xq) + emit_xw_quarter(xrr, xq))
            # --- ACT: per-gate activations (z cols [g | f | i | o]) —
            # f right after g so the c-chain (tfc) starts earliest
            gg = gate_pool.tile([BL, 512], BF16, tag="gg")
            ns.activation(gg[:], pzs[0][:], AFT.Tanh)
            sf = gate_pool.tile([BL, 512], BF16, tag="sf")
            ns.activation(sf[:], pzs[1][:], AFT.Sigmoid)
            si = gate_pool.tile([BL, 512], BF16, tag="si")
            ns.activation(si[:], pzs[2][:], AFT.Sigmoid)
            so = gate_pool.tile([BL, 512], BF16, tag="so")
            ns.activation(so[:], pzs[3][:], AFT.Sigmoid)
            # --- DVE: c update (tfc first — it only needs sf and c)
            if t == 0:
                nv.tensor_tensor(c_sb[:], si[:], gg[:], op=ALU.mult)
            else:
                tfc = tmp_pool.tile([BL, H], BF16, tag="tfc")
                nv.tensor_tensor(tfc[:], sf[:], c_sb[:], op=ALU.mult)
                tig = tmp_pool.tile([BL, H], BF16, tag="tig")
                nv.tensor_tensor(tig[:], si[:], gg[:], op=ALU.mult)
                nv.tensor_tensor(c_sb[:], tfc[:], tig[:], op=ALU.add)
            # --- PE: transposes of sig_o and c into one PSUM tile
            psOC = ps_oc.tile([128, 256], BF16, tag="psOC")
            for c4 in range(4):
                nt.transpose(psOC[:, 32 * c4:32 * (c4 + 1)],
                             so[:, 128 * c4:128 * (c4 + 1)], i32s_bf[0:32, :])
            for c4 in range(4):
                nt.transpose(psOC[:, 128 + 32 * c4:128 + 32 * (c4 + 1)],
                             c_sb[:, 128 * c4:128 * (c4 + 1)], i32s_bf[0:32, :])
            # --- PE: injects for t+1 (fill the tail stall, keep PE warm)
            if t + 1 < t_steps:
                emit_injects(t + 1)
            # --- ACT: tanh on transposed c (128-partition, short free dim)
            tcT = tct_pool.tile([128, 128], BF16, tag="tcT")
            ns.activation(tcT[:], psOC[:, 128:256], AFT.Tanh)
            # --- DVE: h8 = fp8(sig_oT * tanh_cT) halves feed next whh first,
            # then the fp32r hsT copy for pooling/a2 (off the critical path)
            h8 = h8_pool.tile([128, 128], F8E4, tag="h8")
            nv.tensor_tensor(h8[:], psOC[:, 0:128], tcT[:], op=ALU.mult)
            h8_prev = h8
            nv.tensor_tensor(hsT[:, t * 128:(t + 1) * 128],
                             psOC[:, 0:128], tcT[:], op=ALU.mult)
            # --- queue-tail copies: xw quarters + a1
            for rr, q, pxw_pend, pa1_pend in xw_pends:
                if q % 2 == 0:
                    ns.copy(xw_tiles[rr][:, 512 * q:512 * (q + 1)],
                            pxw_pend[:])
                else:
                    nv.tensor_copy(xw_tiles[rr][:, 512 * q:512 * (q + 1)],
                                   pxw_pend[:])
                if pa1_pend is not None:
                    ns.copy(a1ch[:, rr:rr + 1], pa1_pend[:, 0:1])
            xw_tiles.pop(r - 1, None)
            xtc_tiles.pop(r - 1, None)

        loop_ctx.close()

        # ---- post-loop: batched a2 = u2 . h_t over all (b, t) ----
        a2ctx = ExitStack()
        ps_a2b = a2ctx.enter_context(tc.tile_pool(name="ps_a2b", bufs=8, space="PSUM"))
        a2stg = a2ctx.enter_context(tc.tile_pool(name="a2stg", bufs=8))
        hsT_bt = hsT[:].rearrange("p (t c b) -> p c b t", c=4, b=BL)
        for j in range(8):
            pa2b = ps_a2b.tile([1, 512], F32, tag="pa2b")
            for c4 in range(4):
                nt.matmul(pa2b[:], u2t_sb[:, 2 * c4:2 * c4 + 1],
                          hsT_bt[:, c4, 4 * j:4 * (j + 1), :],
                          start=(c4 == 0), stop=(c4 == 3))
            a2r = a2stg.tile([1, 512], F32, tag="a2r")
            if j % 2 == 0:
                ns.copy(a2r[:], pa2b[:])
            else:
                nv.tensor_copy(a2r[:], pa2b[:])
            for bb in range(4):
                nc.sync.dma_start(a2_sb[4 * j + bb:4 * j + bb + 1, :],
                                  a2r[0:1, 128 * bb:128 * (bb + 1)])
        a2ctx.close()

        # ---- post-loop: attention scores + softmax + pooling ----
        post = ctx.enter_context(tc.tile_pool(name="post", bufs=1))
        ps_t = ctx.enter_context(tc.tile_pool(name="ps_t", bufs=2, space="PSUM"))
        ps_pool = ctx.enter_context(tc.tile_pool(name="ps_pool", bufs=4, space="PSUM"))
        stg_pool = ctx.enter_context(tc.tile_pool(name="stg", bufs=4))
        hsb_pool = ctx.enter_context(tc.tile_pool(name="hsb", bufs=2))

        # a1 assembly: a1bp[b, 4r+c] = a1ch[32c+b, r]
        a1bp = post.tile([BL, t_steps], F32, tag="a1bp")
        for c in range(4):
            nv.tensor_copy(a1bp[:].rearrange("b (r c) -> b r c", c=4)[:, :, c],
                           a1ch[32 * c:32 * (c + 1), :])
        abp = post.tile([BL, t_steps], F32, tag="abp")
        nv.tensor_tensor(abp[:], a1bp[:], a2_sb[:], op=ALU.add)
        ns.activation(abp[:], abp[:], AFT.Sigmoid)

        # softmax per slot k -> wT [t, 4b+k] (fp32r for the pooling matmul)
        wT = post.tile([t_steps, K * BL], F32R, tag="wT")
        for k in range(K):
            sc = post.tile([BL, t_steps], F32, tag=f"sc{k}")
            nv.tensor_tensor(sc[:], abp[:],
                             maskneg_sb[:, t_steps * k:t_steps * (k + 1)], op=ALU.add)
            mneg = post.tile([BL, 1], F32, tag=f"mneg{k}")
            nv.tensor_reduce(mneg[:], sc[:], axis=mybir.AxisListType.X,
                             op=ALU.max, negate=True)
            ek = post.tile([BL, t_steps], F32, tag=f"ek{k}")
            sk = post.tile([BL, 1], F32, tag=f"sk{k}")
            ns.activation(ek[:], sc[:], AFT.Exp, bias=mneg[:], accum_out=sk[:])
            rk = post.tile([BL, 1], F32, tag=f"rk{k}")
            nv.reciprocal(rk[:], sk[:])
            wk = post.tile([BL, t_steps], F32, tag=f"wk{k}")
            nv.tensor_scalar(out=wk[:], in0=ek[:], scalar1=rk[:],
                             scalar2=valid_sb[:, k:k + 1], op0=ALU.mult, op1=ALU.mult)
            # transpose into wT columns k::4  (wT[t, 4b+k])
            pwT = ps_t.tile([128, 32], F32, tag="pwT")
            nt.transpose(pwT[0:t_steps, :], wk[:], i32s_f[0:32, :])
            nv.tensor_copy(wT[:].rearrange("t (b k) -> t b k", k=4)[:, :, k],
                           pwT[0:t_steps, :])

        # pooling: per b, rebuild hs_b [t, h] via 4 PE transposes, then [4,T]@[T,H]
        hsT_r = hsT[:].rearrange("p (t c b) -> p t c b", c=4, b=BL)
        for b in range(BL):
            hsb = hsb_pool.tile([t_steps, H], F32R, tag="hsb")
            for c in range(4):
                pt = ps_t.tile([128, 128], F32R, tag="pt")
                nt.transpose(pt[0:t_steps, :], hsT_r[:, :, c, b], i128_r[:])
                if c % 2 == 0:
                    ns.copy(hsb[:, 128 * c:128 * (c + 1)], pt[0:t_steps, :])
                else:
                    nv.tensor_copy(hsb[:, 128 * c:128 * (c + 1)], pt[0:t_steps, :])
            pp = ps_pool.tile([K, H], F32, tag="pp")
            nt.matmul(pp[:], wT[0:t_steps, 4 * b:4 * (b + 1)], hsb[:],
                      start=True, stop=True)
            so = stg_pool.tile([K, H], F32, tag="so")
            ns.copy(so[:], pp[:])
            nc.sync.dma_start(d_out[K * b:K * (b + 1), :], so[:])

    nc.compile()
    return nc


def _host_prep(x, W_ih, W_hh, b_ih, b_hh, A1, A2, v1, lengths, label_len):
    assert int(label_len) == K
    # torch gate rows (i,f,g,o) -> z column order (g,f,i,o)
    perm = np.concatenate([np.arange(1024, 1536), np.arange(512, 1024),
                           np.arange(0, 512), np.arange(1536, 2048)])
    wih = np.ascontiguousarray(W_ih[perm].T, dtype=np.float32)          # [256, 2048]
    whhT = np.asarray(W_hh[perm].T, dtype=np.float32)                   # [512, 2048]
    # fp8 DoubleRow pair layout: whh8[p, pr, ks, n] = WhhT[256*pr+128*ks+p, n]
    whh8 = whhT.reshape(2, 2, 128, G).transpose(2, 0, 1, 3).reshape(128, 4 * G)
    whh8 = np.ascontiguousarray(whh8).astype(ml_dtypes.float8_e4m3).view(np.uint8)
    biasrow = ((b_ih + b_hh)[perm]).astype(np.float32).reshape(1, G)
    u1 = (v1 @ A1)[0].astype(np.float32)                                # [256]
    u2 = (v1 @ A2)[0].astype(np.float32)                                # [512]
    u1t = np.zeros((128, 4), dtype=np.float32)                          # [128, 4]
    u1t[:, 0] = u1[0:128]
    u1t[:, 2] = u1[128:256]
    u2t = np.zeros((128, 8), dtype=np.float32)                          # [128, 8]
    for c in range(4):
        u2t[:, 2 * c] = u2[128 * c:128 * (c + 1)]
    i32s = np.zeros((128, 32), dtype=np.float32)
    i32s[np.arange(128), np.arange(128) % 32] = 1.0
    i128 = np.eye(128, dtype=np.float32)

    shared = dict(wih=wih, whh8=whh8, biasrow=biasrow, u1t=u1t, u2t=u2t,
                  i32s=i32s, i128=i128, onesrow=np.ones((1, 128), dtype=np.float32))

    in_maps = []
    for cidx in range(NC):
        sl = slice(cidx * BL, (cidx + 1) * BL)
        xc = x[:, sl, :]                                                # [T, 32, D]
        xT = np.ascontiguousarray(xc.reshape(T * BL, D).T, dtype=np.float32)
        ln = lengths[sl].astype(np.int64)
        t_start = np.maximum(ln - K, 0)
        t_k = t_start[:, None] + np.arange(K)[None, :]                  # [32, 4]
        valid = (t_k <= (ln[:, None] - 1))                              # [32, 4]
        tt = np.arange(T)
        mask = (tt[None, None, :] <= t_k[:, :, None]) & valid[:, :, None]  # [b, k, t]
        maskneg = np.where(mask, 0.0, NEG_INF).astype(np.float32)
        maskneg = np.ascontiguousarray(maskneg.reshape(BL, K * T))      # k-major cols
        in_maps.append(dict(shared, xT=xT, maskneg=maskneg,
                            valid=valid.astype(np.float32)))
    return in_maps


def kernel(**inputs) -> np.ndarray:
    inputs = {k: np.asarray(v) if not np.isscalar(v) else v for k, v in inputs.items()}
    in_maps = _host_prep(**inputs)
    if "nc" not in _cached:
        _cached["nc"] = _build_program()
    nc = _cached["nc"]
    res = run_bass_kernel_spmd(nc, in_maps, core_ids=list(range(NC)))
    outs = []
    for cidx in range(NC):
        o = res.results[cidx]["out"]                                    # [128, 512]
        outs.append(o.reshape(BL, K, H))
    return np.concatenate(outs, axis=0).astype(np.float32)              # [256, 4, 512]


# revision 29
# speedup vs baseline: 2.0749x; 1.0088x over previous
"""Trainium2 Bass kernel for DUPN-style LSTM + windowed-softmax attention pooling.

Math (per batch element b):
  LSTM over T=128 steps (torch gate order), hidden H=512, input D=256.
  a[t] = sigmoid(x[t]·u1 + h[t]·u2), u1 = (v1@A1)^T, u2 = (v1@A2)^T  (folded)
  out[b,k,:] = softmax-pooled sum of h[t] over window t <= t_k, for 4 slots.

Sharding: data-parallel over batch, 32 per core x 8 cores, weights replicated.

Per-core schedule (v2 — pipelined tail, ~7.5us/step target):
  - z gate-column order is (g, i, f, o) so the c-update chain starts as soon
    as the first whh column chunks finish; per-gate [32,512] activations
    overlap the remaining whh matmuls.
  - h is kept ONLY in transposed form: sig(o) and c are PE-transposed
    separately into one PSUM tile, tanh runs on the 128-partition cT, and a
    DVE multiply writes hsT (fp32r) directly — no scalar-copy on the
    recurrence critical path.
  - xw = x@W_ih^T + bias is computed in per-step quarters ([128,512] PSUM,
    3 matmuls each) so the PE prefetch work is spread evenly; quarter
    copies alternate scalar/vector engines at queue tails.
  - a2[t] = h_t·u2 via 4 tiny stationary-reuse matmuls (N=2) during step
    t+1; a1 via u1 matmuls on the xw input chunks.
  - next-step xw injects (identity matmuls, start=True) are emitted between
    the whh block and the transposes to fill the PE stall while the tail
    chain runs, keeping the PE p-state warm.
  - PSUM: pz 4 banks + pxw 1 + psOC 1 + pa1 1 + pa2 1 = 8.
  - Post-loop: windowed softmax with host-built masks, pooling via per-b
    [4,T]@[T,H] matmuls on PE-transposed hs.
"""
import sys

if "/opt/trn_rl_repo" not in sys.path:
    sys.path.insert(0, "/opt/trn_rl_repo")

import numpy as np
import ml_dtypes
import concourse.bass as bass
import concourse.bacc as bacc
import concourse.tile as tile
from concourse import mybir
from concourse.bass_utils import run_bass_kernel_spmd
from contextlib import ExitStack

F32 = mybir.dt.float32
F32R = mybir.dt.float32r
F8E4 = mybir.dt.float8e4
BF16 = mybir.dt.bfloat16
U8 = mybir.dt.uint8
AFT = mybir.ActivationFunctionType
ALU = mybir.AluOpType
DROW = mybir.MatmulPerfMode.DoubleRow

T, BF, D, H, K, NC = 128, 256, 256, 512, 4, 8
BL = BF // NC          # 32 batch per core
G = 4 * H              # 2048
NEG_INF = -1e9

_cached = {}


def _build_program(t_steps=T):
    nc = bacc.Bacc()
    # ---- DRAM I/O (fp32r where feeding matmuls; same bytes as fp32) ----
    d_xT = nc.declare_dram_parameter("xT", [D, t_steps * BL], F32R, isOutput=False)
    d_wih = nc.declare_dram_parameter("wih", [D, G], F32R, isOutput=False)
    # W_hh^T in fp8e4, DoubleRow pair layout [p, pair, ksub, n] flattened
    d_whh8 = nc.declare_dram_parameter("whh8", [128, 2 * 2 * G], U8, isOutput=False)
    d_biasrow = nc.declare_dram_parameter("biasrow", [1, G], F32R, isOutput=False)
    d_ones = nc.declare_dram_parameter("onesrow", [1, 128], F32R, isOutput=False)
    d_u1t = nc.declare_dram_parameter("u1t", [128, 2 * (D // 128)], F32R, isOutput=False)
    d_u2t = nc.declare_dram_parameter("u2t", [128, 2 * (H // 128)], F32R, isOutput=False)
    d_i32s = nc.declare_dram_parameter("i32s", [128, 32], F32, isOutput=False)
    d_i128 = nc.declare_dram_parameter("i128", [128, 128], F32, isOutput=False)
    d_maskneg = nc.declare_dram_parameter("maskneg", [BL, K * t_steps], F32, isOutput=False)
    d_valid = nc.declare_dram_parameter("valid", [BL, K], F32, isOutput=False)
    d_out = nc.declare_dram_parameter("out", [BL * K, H], F32, isOutput=True)

    NRC = t_steps // 4     # row chunks of 128 rows (4 timesteps each)

    with tile.TileContext(nc) as tc, ExitStack() as ctx:
        nv, ns, nt = nc.vector, nc.scalar, nc.tensor

        consts = ctx.enter_context(tc.tile_pool(name="consts", bufs=1))
        big = ctx.enter_context(tc.tile_pool(name="big", bufs=1))

        # ---- load constants ----
        wih_sb = [consts.tile([128, G], F32R, tag=f"wih{i}", name=f"wih{i}")
                  for i in range(2)]
        for i in range(2):
            nc.sync.dma_start(wih_sb[i][:], d_wih[128 * i:128 * (i + 1), :])
        whh8_sb = consts.tile([128, 2 * 2 * G], F8E4, tag="whh8")
        nc.sync.dma_start(whh8_sb[:], d_whh8[:].bitcast(F8E4))
        whh8_r = whh8_sb[:].rearrange("p (pr ks n) -> p pr ks n", pr=2, ks=2)
        biasrow_sb = consts.tile([1, G], F32R, tag="biasrow")
        nc.sync.dma_start(biasrow_sb[:], d_biasrow[:])
        ones_sb = consts.tile([1, 128], F32R, tag="ones")
        nc.sync.dma_start(ones_sb[:], d_ones[:])
        u1t_sb = consts.tile([128, 4], F32R, tag="u1t")
        nc.sync.dma_start(u1t_sb[:], d_u1t[:])
        u2t_sb = consts.tile([128, 8], F32R, tag="u2t")
        nc.sync.dma_start(u2t_sb[:], d_u2t[:])
        i32s_r = consts.tile([128, 32], F32R, tag="i32s_r")
        nc.sync.dma_start(i32s_r[:], d_i32s[:].bitcast(F32R))
        i32s_f = consts.tile([128, 32], F32, tag="i32s_f")
        nc.sync.dma_start(i32s_f[:], d_i32s[:])
        i32s_bf = consts.tile([128, 32], BF16, tag="i32s_bf")
        ns.copy(i32s_bf[:], i32s_f[:])
        i128_r = consts.tile([128, 128], F32R, tag="i128_r")
        nc.sync.dma_start(i128_r[:], d_i128[:].bitcast(F32R))
        maskneg_sb = consts.tile([BL, K * t_steps], F32, tag="maskneg")
        nc.sync.dma_start(maskneg_sb[:], d_maskneg[:])
        valid_sb = consts.tile([BL, K], F32, tag="valid")
        nc.sync.dma_start(valid_sb[:], d_valid[:])

        # ---- persistent state ----
        hsT = big.tile([128, t_steps * 128], F32R, tag="hsT")      # [p, t*128+c*32+b]
        c_sb = big.tile([BL, H], BF16, tag="c")
        a1ch = big.tile([128, NRC], F32, tag="a1ch")               # a1 by row-chunk
        a2_sb = big.tile([BL, t_steps], F32, tag="a2")

        # ---- loop pools ----
        loop_ctx = ExitStack()
        xt_pool = loop_ctx.enter_context(tc.tile_pool(name="xt", bufs=4))
        xw_pool = loop_ctx.enter_context(tc.tile_pool(name="xw", bufs=4))
        gate_pool = loop_ctx.enter_context(tc.tile_pool(name="gate", bufs=2))
        tmp_pool = loop_ctx.enter_context(tc.tile_pool(name="tmp", bufs=2))
        tct_pool = loop_ctx.enter_context(tc.tile_pool(name="tct", bufs=2))
        h8_pool = loop_ctx.enter_context(tc.tile_pool(name="h8", bufs=2))
        ps_z = loop_ctx.enter_context(tc.tile_pool(name="ps_z", bufs=1, space="PSUM"))
        ps_xw = loop_ctx.enter_context(tc.tile_pool(name="ps_xw", bufs=1, space="PSUM"))
        ps_oc = loop_ctx.enter_context(tc.tile_pool(name="ps_oc", bufs=1, space="PSUM"))
        ps_a1 = loop_ctx.enter_context(tc.tile_pool(name="ps_a1", bufs=1, space="PSUM"))

        xw_tiles, xtc_tiles, pz_tiles = {}, {}, {}

        def emit_xw_quarter(rr, q):
            """PE matmuls for xw chunk rr, column quarter q -> pending PSUM.

            Returns (pxw, pa1): caller emits the PSUM->SBUF copies at its
            chosen queue positions. pa1 is non-None only at q==3.
            """
            if q == 0:
                xtc = [xt_pool.tile([128, 128], F32R, tag=f"xtc{kd}",
                                    name=f"xtc{kd}_{rr}") for kd in range(2)]
                for kd in range(2):
                    nc.sync.dma_start(
                        xtc[kd][:],
                        d_xT[128 * kd:128 * (kd + 1), 128 * rr:128 * (rr + 1)])
                xtc_tiles[rr] = xtc
                xw_tiles[rr] = xw_pool.tile([128, G], F32R, tag="xw",
                                            name=f"xw{rr}")
            xtc = xtc_tiles[rr]
            pxw = ps_xw.tile([128, 512], F32, tag="pxw")
            for kd in range(2):
                nt.matmul(pxw[:], xtc[kd],
                          wih_sb[kd][:, 512 * q:512 * (q + 1)],
                          start=(kd == 0), stop=False)
            nt.matmul(pxw[:], ones_sb[:], biasrow_sb[:, 512 * q:512 * (q + 1)],
                      start=False, stop=True)
            pa1 = None
            if q == 3:
                pa1 = ps_a1.tile([128, 2], F32, tag="pa1")
                for kd in range(2):
                    nt.matmul(pa1[:], xtc[kd], u1t_sb[:, 2 * kd:2 * kd + 2],
                              start=(kd == 0), stop=(kd == 1))
            return pxw, pa1

        def emit_injects(t):
            """Identity matmuls seeding pz[t] with xw rows (+bias).

            One PSUM tile PER GATE so each bank's accumulation group closes
            independently — readers (per-gate activations) otherwise wait
            for the whole tile's group, serializing the tail after all whh.
            """
            pzs = [ps_z.tile([BL, 512], F32, tag=f"pz{n}", name=f"pz{n}_{t}")
                   for n in range(4)]
            pz_tiles[t] = pzs
            rn, tn4 = divmod(t, 4)
            xwn = xw_tiles[rn]
            last = (t == 0)   # t=0 has no whh accumulation
            for n in range(4):
                nt.matmul(pzs[n][:],
                          i32s_r[32 * tn4:32 * (tn4 + 1), :],
                          xwn[32 * tn4:32 * (tn4 + 1), 512 * n:512 * (n + 1)],
                          start=True, stop=last,
                          tile_position=(32 * tn4, 0))

        # ---- preamble: xw chunks 0-2 + chunk 3 q0-q2, injects for t=0 ----
        for rr in range(3):
            for q in range(4):
                pxw, pa1 = emit_xw_quarter(rr, q)
                if q % 2 == 0:
                    ns.copy(xw_tiles[rr][:, 512 * q:512 * (q + 1)], pxw[:])
                else:
                    nv.tensor_copy(xw_tiles[rr][:, 512 * q:512 * (q + 1)], pxw[:])
                if pa1 is not None:
                    ns.copy(a1ch[:, rr:rr + 1], pa1[:, 0:1])
        pxw, _ = emit_xw_quarter(3, 0)
        ns.copy(xw_tiles[3][:, 0:512], pxw[:])
        emit_injects(0)

        # gate column order: n0=g, n1=i, n2=f, n3=o (host perm matches)
        for t in range(t_steps):
            r, t4 = divmod(t, 4)
            pzs = pz_tiles.pop(t)
            # --- PE: whh accumulation, fp8 DoubleRow (K=256 per pass),
            # n-outer so gate chunks finish early
            if t > 0:
                h8p = h8_prev[:].rearrange("p (pr ks b) -> p pr ks b",
                                           pr=2, ks=2)
                for n in range(4):
                    for pr in range(2):
                        nt.matmul(
                            pzs[n][:], h8p[:, pr],
                            whh8_r[:, pr, :, 512 * n:512 * (n + 1)],
                            start=False, stop=(pr == 1), perf_mode=DROW)
            # --- PE: xw prefetch, one quarter per iteration, TWO CHUNKS
            # ahead (quarter q of chunk rr at t = 4*rr + q - 13) so the
            # scheduler places its matmuls/copies in earlier steps' slack,
            # never contending with the current step's critical chain.
            xw_pends = []
            xq = (t + 1) % 4
            xrr = (t + 13 - xq) // 4
            if xrr < NRC:
                xw_pends.append((xrr, xq) + emit_xw_quarter(xrr, xq))
            # --- ACT: per-gate activations (z cols [g | f | i | o]) —
            # f right after g so the c-chain (tfc) starts earliest
            gg = gate_pool.tile([BL, 512], BF16, tag="gg")
            ns.activation(gg[:], pzs[0][:], AFT.Tanh)
            sf = gate_pool.tile([BL, 512], BF16, tag="sf")
            ns.activation(sf[:], pzs[1][:], AFT.Sigmoid)
            si = gate_pool.tile([BL, 512], BF16, tag="si")
            ns.activation(si[:], pzs[2][:], AFT.Sigmoid)
            so = gate_pool.tile([BL, 512], BF16, tag="so")
            ns.activation(so[:], pzs[3][:], AFT.Sigmoid)
            # --- DVE: c update (tfc first — it only needs sf and c)
            if t == 0:
                nv.tensor_tensor(c_sb[:], si[:], gg[:], op=ALU.mult)
            else:
                tfc = tmp_pool.tile([BL, H], BF16, tag="tfc")
                nv.tensor_tensor(tfc[:], sf[:], c_sb[:], op=ALU.mult)
                tig = tmp_pool.tile([BL, H], BF16, tag="tig")
                nv.tensor_tensor(tig[:], si[:], gg[:], op=ALU.mult)
                nv.tensor_tensor(c_sb[:], tfc[:], tig[:], op=ALU.add)
            # --- PE: transposes of sig_o and c into one PSUM tile
            psOC = ps_oc.tile([128, 256], BF16, tag="psOC")
            for c4 in range(4):
                nt.transpose(psOC[:, 32 * c4:32 * (c4 + 1)],
                             so[:, 128 * c4:128 * (c4 + 1)], i32s_bf[0:32, :])
            for c4 in range(4):
                nt.transpose(psOC[:, 128 + 32 * c4:128 + 32 * (c4 + 1)],
                             c_sb[:, 128 * c4:128 * (c4 + 1)], i32s_bf[0:32, :])
            # --- PE: injects for t+1 (fill the tail stall, keep PE warm)
            if t + 1 < t_steps:
                emit_injects(t + 1)
            # --- ACT: tanh on transposed c (128-partition, short free dim)
            tcT = tct_pool.tile([128, 128], BF16, tag="tcT")
            ns.activation(tcT[:], psOC[:, 128:256], AFT.Tanh)
            # --- DVE: h8 = fp8(sig_oT * tanh_cT) halves feed next whh first,
            # then the fp32r hsT copy for pooling/a2 (off the critical path)
            h8 = h8_pool.tile([128, 128], F8E4, tag="h8")
            nv.tensor_tensor(h8[:], psOC[:, 0:128], tcT[:], op=ALU.mult)
            h8_prev = h8
            nv.tensor_tensor(hsT[:, t * 128:(t + 1) * 128],
                             psOC[:, 0:128], tcT[:], op=ALU.mult)
            # --- queue-tail copies: xw quarters + a1
            for rr, q, pxw_pend, pa1_pend in xw_pends:
                if q % 2 == 0:
                    ns.copy(xw_tiles[rr][:, 512 * q:512 * (q + 1)],
                            pxw_pend[:])
                else:
                    nv.tensor_copy(xw_tiles[rr][:, 512 * q:512 * (q + 1)],
                                   pxw_pend[:])
                if pa1_pend is not None:
                    ns.copy(a1ch[:, rr:rr + 1], pa1_pend[:, 0:1])
            xw_tiles.pop(r - 3, None)
            xtc_tiles.pop(r - 3, None)

        loop_ctx.close()

        # ---- post-loop: batched a2 = u2 . h_t over all (b, t) ----
        a2ctx = ExitStack()
        ps_a2b = a2ctx.enter_context(tc.tile_pool(name="ps_a2b", bufs=8, space="PSUM"))
        a2stg = a2ctx.enter_context(tc.tile_pool(name="a2stg", bufs=8))
        hsT_bt = hsT[:].rearrange("p (t c b) -> p c b t", c=4, b=BL)
        for j in range(8):
            pa2b = ps_a2b.tile([1, 512], F32, tag="pa2b")
            for c4 in range(4):
                nt.matmul(pa2b[:], u2t_sb[:, 2 * c4:2 * c4 + 1],
                          hsT_bt[:, c4, 4 * j:4 * (j + 1), :],
                          start=(c4 == 0), stop=(c4 == 3))
            a2r = a2stg.tile([1, 512], F32, tag="a2r")
            if j % 2 == 0:
                ns.copy(a2r[:], pa2b[:])
            else:
                nv.tensor_copy(a2r[:], pa2b[:])
            for bb in range(4):
                nc.sync.dma_start(a2_sb[4 * j + bb:4 * j + bb + 1, :],
                                  a2r[0:1, 128 * bb:128 * (bb + 1)])
        a2ctx.close()

        # ---- post-loop: attention scores + softmax + pooling ----
        post = ctx.enter_context(tc.tile_pool(name="post", bufs=1))
        ps_t = ctx.enter_context(tc.tile_pool(name="ps_t", bufs=2, space="PSUM"))
        ps_pool = ctx.enter_context(tc.tile_pool(name="ps_pool", bufs=4, space="PSUM"))
        stg_pool = ctx.enter_context(tc.tile_pool(name="stg", bufs=4))
        hsb_pool = ctx.enter_context(tc.tile_pool(name="hsb", bufs=2))

        # a1 assembly: a1bp[b, 4r+c] = a1ch[32c+b, r]
        a1bp = post.tile([BL, t_steps], F32, tag="a1bp")
        for c in range(4):
            nv.tensor_copy(a1bp[:].rearrange("b (r c) -> b r c", c=4)[:, :, c],
                           a1ch[32 * c:32 * (c + 1), :])
        abp = post.tile([BL, t_steps], F32, tag="abp")
        nv.tensor_tensor(abp[:], a1bp[:], a2_sb[:], op=ALU.add)
        ns.activation(abp[:], abp[:], AFT.Sigmoid)

        # softmax per slot k -> wT [t, 4b+k] (fp32r for the pooling matmul)
        wT = post.tile([t_steps, K * BL], F32R, tag="wT")
        for k in range(K):
            sc = post.tile([BL, t_steps], F32, tag=f"sc{k}")
            nv.tensor_tensor(sc[:], abp[:],
                             maskneg_sb[:, t_steps * k:t_steps * (k + 1)], op=ALU.add)
            mneg = post.tile([BL, 1], F32, tag=f"mneg{k}")
            nv.tensor_reduce(mneg[:], sc[:], axis=mybir.AxisListType.X,
                             op=ALU.max, negate=True)
            ek = post.tile([BL, t_steps], F32, tag=f"ek{k}")
            sk = post.tile([BL, 1], F32, tag=f"sk{k}")
            ns.activation(ek[:], sc[:], AFT.Exp, bias=mneg[:], accum_out=sk[:])
            rk = post.tile([BL, 1], F32, tag=f"rk{k}")
            nv.reciprocal(rk[:], sk[:])
            wk = post.tile([BL, t_steps], F32, tag=f"wk{k}")
            nv.tensor_scalar(out=wk[:], in0=ek[:], scalar1=rk[:],
                             scalar2=valid_sb[:, k:k + 1], op0=ALU.mult, op1=ALU.mult)
            # transpose into wT columns k::4  (wT[t, 4b+k])
            pwT = ps_t.tile([128, 32], F32, tag="pwT")
            nt.transpose(pwT[0:t_steps, :], wk[:], i32s_f[0:32, :])
            nv.tensor_copy(wT[:].rearrange("t (b k) -> t b k", k=4)[:, :, k],
                           pwT[0:t_steps, :])

        # pooling: per b, rebuild hs_b [t, h] via 4 PE transposes, then [4,T]@[T,H]
        hsT_r = hsT[:].rearrange("p (t c b) -> p t c b", c=4, b=BL)
        for b in range(BL):
            hsb = hsb_pool.tile([t_steps, H], F32R, tag="hsb")
            for c in range(4):
                pt = ps_t.tile([128, 128], F32R, tag="pt")
                nt.transpose(pt[0:t_steps, :], hsT_r[:, :, c, b], i128_r[:])
                if c % 2 == 0:
                    ns.copy(hsb[:, 128 * c:128 * (c + 1)], pt[0:t_steps, :])
                else:
                    nv.tensor_copy(hsb[:, 128 * c:128 * (c + 1)], pt[0:t_steps, :])
            pp = ps_pool.tile([K, H], F32, tag="pp")
            nt.matmul(pp[:], wT[0:t_steps, 4 * b:4 * (b + 1)], hsb[:],
                      start=True, stop=True)
            so = stg_pool.tile([K, H], F32, tag="so")
            ns.copy(so[:], pp[:])
            nc.sync.dma_start(d_out[K * b:K * (b + 1), :], so[:])

    nc.compile()
    return nc


def _host_prep(x, W_ih, W_hh, b_ih, b_hh, A1, A2, v1, lengths, label_len):
    assert int(label_len) == K
    # torch gate rows (i,f,g,o) -> z column order (g,f,i,o)
    perm = np.concatenate([np.arange(1024, 1536), np.arange(512, 1024),
                           np.arange(0, 512), np.arange(1536, 2048)])
    wih = np.ascontiguousarray(W_ih[perm].T, dtype=np.float32)          # [256, 2048]
    whhT = np.asarray(W_hh[perm].T, dtype=np.float32)                   # [512, 2048]
    # fp8 DoubleRow pair layout: whh8[p, pr, ks, n] = WhhT[256*pr+128*ks+p, n]
    whh8 = whhT.reshape(2, 2, 128, G).transpose(2, 0, 1, 3).reshape(128, 4 * G)
    whh8 = np.ascontiguousarray(whh8).astype(ml_dtypes.float8_e4m3).view(np.uint8)
    biasrow = ((b_ih + b_hh)[perm]).astype(np.float32).reshape(1, G)
    u1 = (v1 @ A1)[0].astype(np.float32)                                # [256]
    u2 = (v1 @ A2)[0].astype(np.float32)                                # [512]
    u1t = np.zeros((128, 4), dtype=np.float32)                          # [128, 4]
    u1t[:, 0] = u1[0:128]
    u1t[:, 2] = u1[128:256]
    u2t = np.zeros((128, 8), dtype=np.float32)                          # [128, 8]
    for c in range(4):
        u2t[:, 2 * c] = u2[128 * c:128 * (c + 1)]
    i32s = np.zeros((128, 32), dtype=np.float32)
    i32s[np.arange(128), np.arange(128) % 32] = 1.0
    i128 = np.eye(128, dtype=np.float32)

    shared = dict(wih=wih, whh8=whh8, biasrow=biasrow, u1t=u1t, u2t=u2t,
                  i32s=i32s, i128=i128, onesrow=np.ones((1, 128), dtype=np.float32))

    in_maps = []
    for cidx in range(NC):
        sl = slice(cidx * BL, (cidx + 1) * BL)
        xc = x[:, sl, :]                                                # [T, 32, D]
        xT = np.ascontiguousarray(xc.reshape(T * BL, D).T, dtype=np.float32)
        ln = lengths[sl].astype(np.int64)
        t_start = np.maximum(ln - K, 0)
        t_k = t_start[:, None] + np.arange(K)[None, :]                  # [32, 4]
        valid = (t_k <= (ln[:, None] - 1))                              # [32, 4]
        tt = np.arange(T)
        mask = (tt[None, None, :] <= t_k[:, :, None]) & valid[:, :, None]  # [b, k, t]
        maskneg = np.where(mask, 0.0, NEG_INF).astype(np.float32)
        maskneg = np.ascontiguousarray(maskneg.reshape(BL, K * T))      # k-major cols
        in_maps.append(dict(shared, xT=xT, maskneg=maskneg,
                            valid=valid.astype(np.float32)))
    return in_maps


def kernel(**inputs) -> np.ndarray:
    inputs = {k: np.asarray(v) if not np.isscalar(v) else v for k, v in inputs.items()}
    in_maps = _host_prep(**inputs)
    if "nc" not in _cached:
        _cached["nc"] = _build_program()
    nc = _cached["nc"]
    res = run_bass_kernel_spmd(nc, in_maps, core_ids=list(range(NC)))
    outs = []
    for cidx in range(NC):
        o = res.results[cidx]["out"]                                    # [128, 512]
        outs.append(o.reshape(BL, K, H))
    return np.concatenate(outs, axis=0).astype(np.float32)              # [256, 4, 512]
